# revision 49
# baseline (speedup 1.0000x reference)
"""TRN2 Bass kernel for nn_BatchDenseGAT (2-layer dense GAT, bs=32, n=512).

Sharding: data-parallel over the 32 graphs -> 4 graphs per NeuronCore x 8
cores, params replicated. Host does embedding gather/concat/transpose and
mask packing; all model math runs on device.

Device strategy (rank-1 attention factorization): for a GAT layer,
  exp(lrelu(s_i + d_j)) * adj[j,i]  ==  v_i * adjT[j,i] * p_j * max(w_j, r_i)
with u=exp(d), p=exp(0.2 d), w=exp(0.8 d), r=exp(-0.8 s), v=exp(s).
The v_i factor is constant along the softmax axis (j) and cancels against
the denominator, so the masked unnormalized weights are
  em'[j,i] = adjT[j,i] * p_j * max(w_j, r_i)
which needs only O(n) exponentials (rows/cols) instead of O(n^2): the n^2
work reduces to one 4x-mode tensor_scalar (max,mult) + one bf16
tensor_tensor (mask) per head, replacing the Prelu+Exp activation passes.

Aggregation runs "flipped" (out[i,o+den] via lhsT=em' blocks, ones-col in
the rhs for the denominator) so the softmax denominator is a per-partition
scalar: normalize rides an ACT copy's per-partition `scale`, elu is
  h2 = exp(min(v,0)) + relu(v) - 1,   v = out1 * rden
with 4x-mode dual-op tensor_scalars, and h2 is transposed back to [f, i]
with PE transposes for layer 2. The "-1" folds into layer-2 weights
(negcs). r broadcasts [128,512] come from one matmul with a constant
one-hot selector lhsT (bc8). Layer 2 repeats the scheme with one head.
log_softmax: per-graph Exp + pairwise-sum partials, one deferred Ln at the
end (single act-table switch; everything else lives in `exp_and_others`),
one batched output DMA.

Engine balance (cost-model tuned): the n^2 ops are split per head across
DVE/Pool (em' mask-mul 3:1 split), ACT takes broadcasts/copies/exps, PE
takes matmuls/transposes. fp32r is avoided (schedule-dependent corruption
on the NEFF path); h' matmuls run in bf16.
"""
import os
import sys
import numpy as np

sys.path.insert(0, '/opt/trn_rl_repo')

import ml_dtypes  # noqa: E402
import concourse.bacc as bacc  # noqa: E402
import concourse.bass as bass  # noqa: E402
import concourse.tile as tile  # noqa: E402
from concourse import mybir  # noqa: E402

F32 = mybir.dt.float32
F32R = mybir.dt.float32r
BF16 = mybir.dt.bfloat16
AF = mybir.ActivationFunctionType
ALU = mybir.AluOpType
BFNP = ml_dtypes.bfloat16

BS, N, NH, FO = 32, 512, 8, 64
HP_DT = {"f32r": mybir.dt.float32r, "f32": mybir.dt.float32,
         "bf16": mybir.dt.bfloat16}[os.environ.get("GAT_HP_DT", "bf16")]
FIN1 = 160
NCORES = 8
GPC = BS // NCORES  # graphs per core
NCH = 4             # 512 / 128 partition chunks

# engine knobs: per-head choices, tuned against TimelineSim.
# (gpsimd/Pool cannot read PSUM, so PSUM->SBUF copies are ACT/DVE only)
# em' mask-multiply engine per head: 'v' = DVE, 'g' = Pool/gpsimd
EM_ENG = os.environ.get("GAT_EM_ENG", "66666666")
# elu path per head: 'a' = ACT copy-scale + 4x DVE min/max,
#                    'v' = DVE dual-op TS direct from PSUM
ELU_ENG = os.environ.get("GAT_ELU_ENG", "aaaaaaaa")
# elu add engine per head: 'v' = DVE, 'g' = Pool
AD_ENG = os.environ.get("GAT_AD_ENG", "vvvvvvvv")
# r_bc copy engine per head: 'a' = ACT, 'v' = DVE
RB_ENG = os.environ.get("GAT_RB_ENG", "aaaaaaaa")
# h2cT copy engine per head: 'v' = DVE, 'a' = ACT
H2_ENG = os.environ.get("GAT_H2_ENG", "aaaavvvv")
# hpx chunk copy engine per ic: 'a' = ACT, 'v' = DVE
HPX_ENG = os.environ.get("GAT_HPX_ENG", "vvvv")
# c-chunk offload per head: '-' = all DVE, 'g' = last chunk on Pool,
# 'G' = last two chunks on Pool
CH_ENG = os.environ.get("GAT_CH_ENG", "gggggggg")
# elu relu-part (p_t) engine per head: 'v' = DVE, 'g' = Pool (off-chain)
MPP_ENG = os.environ.get("GAT_MPP_ENG", "vvvvvvvv")


def build_nc():
    B = 1 if os.environ.get("GAT_SERIAL", "0") == "1" else None
    dbg = os.environ.get("GAT_DEBUG_DUMP", "0") == "1"
    dbg_lvl = int(os.environ.get("GAT_DEBUG_LVL", "3"))
    dbg_g = int(os.environ.get("GAT_DEBUG_G", "0"))
    nc = bacc.Bacc("TRN2", target_bir_lowering=False, debug=False)

    hT4 = nc.dram_tensor("hT4", [GPC, FIN1, N], HP_DT, kind="ExternalInput")
    adjTp = nc.dram_tensor("adjTp", [GPC, 128, NCH * N], BF16,
                           kind="ExternalInput")
    w1f_d = nc.dram_tensor("w1f", [FIN1, 512], HP_DT, kind="ExternalInput")
    asrc_d = nc.dram_tensor("asrc", [512, NH], BF16, kind="ExternalInput")
    adst_d = nc.dram_tensor("adst", [512, NH], BF16, kind="ExternalInput")
    w2f_d = nc.dram_tensor("w2f", [512, 16], BF16, kind="ExternalInput")
    negcs_d = nc.dram_tensor("negcs", [1, 16], BF16, kind="ExternalInput")
    a2s_d = nc.dram_tensor("a2s", [16, 1], BF16, kind="ExternalInput")
    a2d_d = nc.dram_tensor("a2d", [16, 1], BF16, kind="ExternalInput")
    ident_d = nc.dram_tensor("ident", [128, 128], BF16, kind="ExternalInput")
    bc8_d = nc.dram_tensor("bc8", [NH, NH * 128], BF16, kind="ExternalInput")
    out_d = nc.dram_tensor("out", [GPC, N, 16], F32, kind="ExternalOutput")
    if dbg:
        dbg_tT = nc.dram_tensor("dbg_tT", [128, NCH, 512], BF16,
                                kind="ExternalOutput")
        dbg_rT = nc.dram_tensor("dbg_rT", [1, NH * 512], BF16,
                                kind="ExternalOutput")
        dbg_wp = nc.dram_tensor("dbg_wp", [128, 2, NCH, NH], F32,
                                kind="ExternalOutput")
        dbg_em = nc.dram_tensor("dbg_em", [128, NCH * 512], BF16,
                                kind="ExternalOutput")
        dbg_h2v = nc.dram_tensor("dbg_h2v", [128, NCH, 64], BF16,
                                 kind="ExternalOutput")
        dbg_h2cT = nc.dram_tensor("dbg_h2cT", [128, NCH, 512], BF16,
                                  kind="ExternalOutput")
        dbg_rden = nc.dram_tensor("dbg_rden", [128, NCH], F32,
                                  kind="ExternalOutput")
        dbg_lg = nc.dram_tensor("dbg_lg", [128, NCH, 16], F32,
                                kind="ExternalOutput")

    with tile.TileContext(nc) as tc:
        with tc.tile_pool(name="consts", bufs=1) as consts, \
             tc.tile_pool(name="gbuf", bufs=(B or int(os.environ.get("GAT_GBUF", "3")))) as gbuf, \
             tc.tile_pool(name="attn", bufs=(B or int(os.environ.get("GAT_ATTN", "3")))) as attn, \
             tc.tile_pool(name="small", bufs=(B or 2)) as small, \
             tc.tile_pool(name="elu", bufs=(B or int(os.environ.get("GAT_ELUB", "3")))) as elup, \
             tc.tile_pool(name="fin", bufs=1) as finp, \
             tc.tile_pool(name="ps_h", bufs=(B or int(os.environ.get("GAT_PSH", "1"))), space="PSUM") as ps_h, \
             tc.tile_pool(name="ps_rb", bufs=(B or 1), space="PSUM") as ps_rb, \
             tc.tile_pool(name="ps_sT", bufs=(B or 1), space="PSUM") as ps_sT, \
             tc.tile_pool(name="ps_row", bufs=(B or 1), space="PSUM") as ps_row, \
             tc.tile_pool(name="ps_o", bufs=(B or int(os.environ.get("GAT_PSO", "2"))), space="PSUM") as ps_o, \
             tc.tile_pool(name="ps_t", bufs=(B or 1), space="PSUM") as ps_t:

            # ---------- constants ----------
            w1f_a = consts.tile([128, 512], HP_DT)
            w1f_b = consts.tile([32, 512], HP_DT)
            nc.sync.dma_start(out=w1f_a, in_=w1f_d.ap()[0:128, :])
            nc.sync.dma_start(out=w1f_b, in_=w1f_d.ap()[128:160, :])

            # prefetch graph-0 loads ahead of the small consts: HWDGE is
            # serial (~625ns/DMA) and these gate the first matmuls.
            g_order = list(range(GPC))
            if os.environ.get("GAT_REV", "0") == "1":
                g_order = g_order[::-1]
            g0 = g_order[0]
            pref_hT_a = gbuf.tile([128, N], HP_DT, tag="hT_a")
            pref_hT_b = gbuf.tile([32, N], HP_DT, tag="hT_b")
            pref_adjT = gbuf.tile([128, NCH * N], BF16, tag="adjT")
            pref = {"hT_a": pref_hT_a, "hT_b": pref_hT_b, "adjT": pref_adjT}
            nc.sync.dma_start(out=pref_hT_a, in_=hT4.ap()[g0, 0:128, :])
            nc.sync.dma_start(out=pref_hT_b, in_=hT4.ap()[g0, 128:160, :])
            nc.sync.dma_start(out=pref_adjT, in_=adjTp.ap()[g0])

            asrc_sb = consts.tile([128, NCH, NH], BF16)
            adst_sb = consts.tile([128, NCH, NH], BF16)
            nc.sync.dma_start(out=asrc_sb,
                              in_=asrc_d.ap().rearrange("(c p) a -> p c a", c=NCH))
            nc.sync.dma_start(out=adst_sb,
                              in_=adst_d.ap().rearrange("(c p) a -> p c a", c=NCH))
            w2f_sb = consts.tile([128, NCH, 16], BF16)
            nc.sync.dma_start(out=w2f_sb,
                              in_=w2f_d.ap().rearrange("(c p) a -> p c a", c=NCH))
            negcs_sb = consts.tile([1, 16], BF16)
            nc.sync.dma_start(out=negcs_sb, in_=negcs_d.ap())
            a2s_sb = consts.tile([16, 1], BF16)
            a2d_sb = consts.tile([16, 1], BF16)
            nc.sync.dma_start(out=a2s_sb, in_=a2s_d.ap())
            nc.sync.dma_start(out=a2d_sb, in_=a2d_d.ap())
            ident = consts.tile([128, 128], BF16)
            nc.sync.dma_start(out=ident, in_=ident_d.ap())
            bc8 = consts.tile([NH, NH * 128], BF16)
            nc.sync.dma_start(out=bc8, in_=bc8_d.ap())
            onesrow = consts.tile([1, 512], BF16)
            nc.vector.memset(onesrow, 1.0)

            logits_all = finp.tile([128, GPC, NCH, 16], F32)
            s1_all = finp.tile([128, GPC, NCH], F32)

            for g in g_order:
                # ---------- graph loads ----------
                if g == g0:
                    hT_a, hT_b, adjT_sb = (pref["hT_a"], pref["hT_b"],
                                           pref["adjT"])
                else:
                    hT_a = gbuf.tile([128, N], HP_DT, tag="hT_a")
                    hT_b = gbuf.tile([32, N], HP_DT, tag="hT_b")
                    nc.sync.dma_start(out=hT_a, in_=hT4.ap()[g, 0:128, :])
                    nc.sync.dma_start(out=hT_b, in_=hT4.ap()[g, 128:160, :])
                    adjT_sb = gbuf.tile([128, NCH * N], BF16, tag="adjT")
                    nc.sync.dma_start(out=adjT_sb, in_=adjTp.ap()[g])

                # ---------- h_primeT (o-major) -> tanh -> tT bf16 ----------
                # (first: its tanh -> sT -> rT chain gates the heads; oc
                # pairs share one [128,1024] PSUM tile so each tanh is one
                # 1024-wide op)
                tT = gbuf.tile([128, NCH, 512], BF16, tag="tT")
                for op in range(NCH // 2):
                    hpT_ps = ps_h.tile([128, 2, 512], F32, tag="big")
                    for k in range(2):
                        oc = op * 2 + k
                        nc.tensor.matmul(hpT_ps[:, k, :],
                                         w1f_a[:, oc * 128:(oc + 1) * 128],
                                         hT_a[:],
                                         start=True, stop=False)
                        nc.tensor.matmul(hpT_ps[:, k, :],
                                         w1f_b[:, oc * 128:(oc + 1) * 128],
                                         hT_b[:],
                                         start=False, stop=True)
                    nc.scalar.activation(
                        tT[:, op * 2:op * 2 + 2, :].rearrange(
                            "p a b -> p (a b)"),
                        hpT_ps[:].rearrange("p a b -> p (a b)"), AF.Tanh)

                # ---------- sT row [8, 512] -> rT = exp(-0.8 s) ----------
                sT_ps = ps_sT.tile([16, 512], F32, tag="sT")
                for oc in range(NCH):
                    nc.tensor.matmul(sT_ps[0:NH, :], asrc_sb[:, oc, :],
                                     tT[:, oc, :],
                                     start=(oc == 0), stop=(oc == NCH - 1))
                rT = small.tile([NH, 512], BF16, tag="rT")
                nc.scalar.activation(rT[:], sT_ps[0:NH, :], AF.Exp, scale=-0.8)

                # -- h_prime (n-major) -> hpx bf16 [j, (jc), (h, 64+ones)] --
                hpx = gbuf.tile([128, NCH, NH, 65], BF16, tag="hpx")
                nc.vector.memset(hpx[:, :, :, 64:65], 1.0)
                for ip in range(NCH // 2):
                    hp_ps = ps_h.tile([128, 2, 512], F32, tag="big")
                    for k in range(2):
                        ic = ip * 2 + k
                        nc.tensor.matmul(hp_ps[:, k, :],
                                         hT_a[:, ic * 128:(ic + 1) * 128],
                                         w1f_a[:],
                                         start=True, stop=False)
                        nc.tensor.matmul(hp_ps[:, k, :],
                                         hT_b[:, ic * 128:(ic + 1) * 128],
                                         w1f_b[:],
                                         start=False, stop=True)
                    if HPX_ENG[ip * 2] == 'a':
                        nc.scalar.copy(
                            hpx[:, ip * 2:ip * 2 + 2, :, 0:64],
                            hp_ps[:].rearrange("p c (h o) -> p c h o", h=NH))
                    else:
                        nc.vector.tensor_copy(
                            hpx[:, ip * 2:ip * 2 + 2, :, 0:64],
                            hp_ps[:].rearrange("p c (h o) -> p c h o", h=NH))


                # ---------- d cols -> w = exp(0.8 d), p = exp(0.2 d) ------
                d_ps = ps_o.tile([128, NCH, 65], F32, tag="o65")
                for jc in range(NCH):
                    for oc in range(NCH):
                        nc.tensor.matmul(d_ps[:, jc, 0:NH],
                                         tT[:, oc, jc * 128:(jc + 1) * 128],
                                         adst_sb[:, oc, :],
                                         start=(oc == 0), stop=(oc == NCH - 1))
                w_sb = small.tile([128, NCH, NH], F32, tag="w_sb")
                p_sb = small.tile([128, NCH, NH], F32, tag="p_sb")
                nc.scalar.activation(w_sb[:], d_ps[:, :, 0:NH], AF.Exp,
                                     scale=0.8)
                nc.scalar.activation(p_sb[:], d_ps[:, :, 0:NH], AF.Exp,
                                     scale=0.2)

                if dbg and g == dbg_g and dbg_lvl >= 2:
                    nc.sync.dma_start(out=dbg_tT.ap(), in_=tT[:])
                    nc.sync.dma_start(out=dbg_rT.ap(), in_=rT1[:])
                    nc.sync.dma_start(out=dbg_wp.ap()[:, 0], in_=w_sb[:])
                    nc.sync.dma_start(out=dbg_wp.ap()[:, 1], in_=p_sb[:])

                # ---------- per-head attention ----------
                h2cT = gbuf.tile([128, NCH, 512], BF16, tag="h2cT")
                for h in range(NH):
                    # r broadcast [128, 512] via PE rank-1, copy to bf16 sbuf
                    rbps = ps_rb.tile([128, 512], F32, tag="rb")
                    nc.tensor.matmul(rbps[:],
                                     bc8[:, h * 128:(h + 1) * 128],
                                     rT[:], start=True, stop=True)
                    r_bc = attn.tile([128, 512], BF16, tag="r_bc")
                    if RB_ENG[h] == 'a':
                        nc.scalar.copy(r_bc[:], rbps[:])
                    else:
                        nc.vector.tensor_copy(r_bc[:], rbps[:])

                    # c[j,i] = max(w_j, r_i) * p_j  (4x-mode dual-op TS)
                    c_all = attn.tile([128, NCH, 512], BF16, tag="c_all")
                    n_pool_c = {'-': 0, 'g': 1, 'G': 2}[CH_ENG[h]]
                    for jc in range(NCH):
                        eng = (nc.gpsimd if jc >= NCH - n_pool_c
                               else nc.vector)
                        eng.tensor_scalar(
                            c_all[:, jc, :], r_bc[:],
                            w_sb[:, jc, h:h + 1], p_sb[:, jc, h:h + 1],
                            ALU.max, ALU.mult)
                    # em' = c * adjT (split across DVE/Pool to balance)
                    em_all = attn.tile([128, NCH * 512], BF16, tag="em")
                    cflat = c_all[:].rearrange("p a b -> p (a b)")
                    ch = EM_ENG[h]
                    if ch == 'g':
                        nc.gpsimd.tensor_mul(em_all[:], cflat, adjT_sb[:])
                    elif ch == 'v':
                        nc.vector.tensor_mul(em_all[:], cflat, adjT_sb[:])
                    else:
                        cut = int(ch) * 256
                        nc.vector.tensor_mul(em_all[:, 0:cut],
                                             cflat[:, 0:cut],
                                             adjT_sb[:, 0:cut])
                        nc.gpsimd.tensor_mul(em_all[:, cut:2048],
                                             cflat[:, cut:2048],
                                             adjT_sb[:, cut:2048])

                    # out1 flipped: [i, 64+den] per ic chunk
                    o65v = ps_o.tile([128, NCH, 65], F32, tag="o65")
                    for ic in range(NCH):
                        for jc in range(NCH):
                            nc.tensor.matmul(
                                o65v[:, ic, :],
                                em_all[:, jc * 512 + ic * 128:
                                       jc * 512 + (ic + 1) * 128],
                                hpx[:, jc, h, :],
                                start=(jc == 0), stop=(jc == NCH - 1))
                    rden = elup.tile([128, NCH], F32, tag="rden")
                    nc.vector.reciprocal(rden[:], o65v[:, :, 64])

                    # elu: m = min(v,0), p = relu(v), v = out1*rden
                    m_t = elup.tile([128, NCH, 64], BF16, tag="m_t")
                    p_t = elup.tile([128, NCH, 64], BF16, tag="p_t")
                    if ELU_ENG[h] in 'abd':
                        # fold the per-partition rden into a scaled copy;
                        # min/max are then 4x-mode SBUF tensor_scalars.
                        # 'b': one DVE tensor_tensor with a stride-0
                        # broadcast view of rden (cheapest op count).
                        v_sb = elup.tile([128, NCH, 64], BF16, tag="v_sb")
                        if ELU_ENG[h] == 'b':
                            nc.vector.tensor_mul(
                                v_sb[:], o65v[:, :, 0:64],
                                rden[:].broadcast_to([128, NCH, 64]))
                        else:
                            for ic in range(NCH):
                                if ELU_ENG[h] == 'a':
                                    nc.scalar.activation(
                                        v_sb[:, ic, :], o65v[:, ic, 0:64],
                                        AF.Copy, scale=rden[:, ic:ic + 1])
                                else:
                                    nc.vector.tensor_scalar(
                                        v_sb[:, ic, :], o65v[:, ic, 0:64],
                                        rden[:, ic:ic + 1], None, ALU.mult)
                        vf = v_sb[:].rearrange("p a b -> p (a b)")
                        nc.vector.tensor_scalar(
                            m_t[:].rearrange("p a b -> p (a b)"), vf,
                            0.0, None, ALU.min)
                        peng = (nc.gpsimd if MPP_ENG[h] == 'g'
                                else nc.vector)
                        peng.tensor_scalar(
                            p_t[:].rearrange("p a b -> p (a b)"), vf,
                            0.0, None, ALU.max)
                    else:
                        for ic in range(NCH):
                            nc.vector.tensor_scalar(
                                m_t[:, ic, :], o65v[:, ic, 0:64],
                                rden[:, ic:ic + 1], 0.0, ALU.mult, ALU.min)
                            nc.vector.tensor_scalar(
                                p_t[:, ic, :], o65v[:, ic, 0:64],
                                rden[:, ic:ic + 1], 0.0, ALU.mult, ALU.max)
                    em_t = elup.tile([128, NCH, 64], BF16, tag="em_t")
                    nc.scalar.activation(
                        em_t[:].rearrange("p a b -> p (a b)"),
                        m_t[:].rearrange("p a b -> p (a b)"), AF.Exp)
                    h2v = elup.tile([128, NCH, 64], BF16, tag="h2v")
                    if AD_ENG[h] == 'g':
                        nc.gpsimd.tensor_add(
                            h2v[:].rearrange("p a b -> p (a b)"),
                            em_t[:].rearrange("p a b -> p (a b)"),
                            p_t[:].rearrange("p a b -> p (a b)"))
                    else:
                        nc.vector.tensor_add(
                            h2v[:].rearrange("p a b -> p (a b)"),
                            em_t[:].rearrange("p a b -> p (a b)"),
                            p_t[:].rearrange("p a b -> p (a b)"))

                    if dbg and g == dbg_g and h == 0 and dbg_lvl >= 3:
                        nc.sync.dma_start(out=dbg_em.ap(), in_=em_all[:])
                        nc.sync.dma_start(out=dbg_h2v.ap(), in_=h2v[:])
                        nc.sync.dma_start(out=dbg_rden.ap(), in_=rden[:])

                    # transpose h2v [i,(ic),64] -> h2cT rows [64, 512]
                    h2T_ps = ps_t.tile([64, 512], BF16, tag="t")
                    for ic in range(NCH):
                        nc.tensor.transpose(
                            h2T_ps[:, ic * 128:(ic + 1) * 128],
                            h2v[:, ic, :], ident[:])
                    prow = (h % 2) * 64
                    if H2_ENG[h] == 'a':
                        nc.scalar.copy(h2cT[prow:prow + 64, h // 2, :],
                                       h2T_ps[:])
                    else:
                        nc.vector.tensor_copy(
                            h2cT[prow:prow + 64, h // 2, :], h2T_ps[:])

                if dbg and g == dbg_g:
                    nc.sync.dma_start(out=dbg_h2cT.ap(), in_=h2cT[:])

                # ================= layer 2 =================
                # h_prime2 (n-major) [i, 16] + ones col -> hp2x bf16
                hp2_ps = ps_o.tile([128, NCH, 65], F32, tag="o65")
                for ic in range(NCH):
                    for fc in range(NCH):
                        nc.tensor.matmul(hp2_ps[:, ic, 0:16],
                                         h2cT[:, fc, ic * 128:(ic + 1) * 128],
                                         w2f_sb[:, fc, :],
                                         start=(fc == 0), stop=False)
                    nc.tensor.matmul(hp2_ps[:, ic, 0:16],
                                     onesrow[:, ic * 128:(ic + 1) * 128],
                                     negcs_sb[:],
                                     start=False, stop=True)
                hp2x = small.tile([128, NCH, 17], BF16, tag="hp2x")
                nc.vector.tensor_copy(hp2x[:, :, 0:16], hp2_ps[:, :, 0:16])
                nc.vector.memset(hp2x[:, :, 16:17], 1.0)

                # h_prime2T [16, n] -> tanh -> t2 bf16
                hp2T_ps = ps_row.tile([16, 512], F32, tag="row")
                for fc in range(NCH):
                    nc.tensor.matmul(hp2T_ps[:], w2f_sb[:, fc, :],
                                     h2cT[:, fc, :],
                                     start=(fc == 0), stop=False)
                nc.tensor.matmul(hp2T_ps[:], negcs_sb[:], onesrow[:],
                                 start=False, stop=True)
                t2_sb = small.tile([16, 512], BF16, tag="t2")
                nc.scalar.activation(t2_sb[:], hp2T_ps[:], AF.Tanh)

                # s2 row -> r2 = exp(-0.8 s2); d2 cols -> w2c, p2c
                s2_ps = ps_row.tile([16, 512], F32, tag="row")
                nc.tensor.matmul(s2_ps[0:1, :], a2s_sb[:], t2_sb[:],
                                 start=True, stop=True)
                r2 = small.tile([1, 512], BF16, tag="r2")
                nc.scalar.activation(r2[:], s2_ps[0:1, :], AF.Exp, scale=-0.8)
                d2_ps = ps_o.tile([128, NCH, 65], F32, tag="o65")
                for jc in range(NCH):
                    nc.tensor.matmul(d2_ps[:, jc, 0:1],
                                     t2_sb[:, jc * 128:(jc + 1) * 128],
                                     a2d_sb[:], start=True, stop=True)
                w2c = small.tile([128, NCH], F32, tag="w2c")
                p2c = small.tile([128, NCH], F32, tag="p2c")
                nc.scalar.activation(w2c[:], d2_ps[:, :, 0], AF.Exp, scale=0.8)
                nc.scalar.activation(p2c[:], d2_ps[:, :, 0], AF.Exp, scale=0.2)

                # r2 broadcast + c2 + em2
                rb2ps = ps_rb.tile([128, 512], F32, tag="rb")
                nc.tensor.matmul(rb2ps[:], onesrow[:, 0:128], r2[:],
                                 start=True, stop=True)
                r2_bc = attn.tile([128, 512], BF16, tag="r_bc")
                nc.scalar.copy(r2_bc[:], rb2ps[:])
                c2_all = attn.tile([128, NCH, 512], BF16, tag="c_all")
                for jc in range(NCH):
                    nc.vector.tensor_scalar(
                        c2_all[:, jc, :], r2_bc[:],
                        w2c[:, jc:jc + 1], p2c[:, jc:jc + 1],
                        ALU.max, ALU.mult)
                em2_all = attn.tile([128, NCH * 512], BF16, tag="em")
                c2flat = c2_all[:].rearrange("p a b -> p (a b)")
                nc.vector.tensor_mul(em2_all[:, 0:1536], c2flat[:, 0:1536],
                                     adjT_sb[:, 0:1536])
                nc.gpsimd.tensor_mul(em2_all[:, 1536:2048],
                                     c2flat[:, 1536:2048],
                                     adjT_sb[:, 1536:2048])

                # out2 flipped [i, 16+den] per ic; logits = out2 * rden2
                o2v = ps_o.tile([128, NCH, 65], F32, tag="o65")
                for ic in range(NCH):
                    for jc in range(NCH):
                        nc.tensor.matmul(
                            o2v[:, ic, 0:17],
                            em2_all[:, jc * 512 + ic * 128:
                                    jc * 512 + (ic + 1) * 128],
                            hp2x[:, jc, :],
                            start=(jc == 0), stop=(jc == NCH - 1))
                rden2 = elup.tile([128, NCH], F32, tag="rden")
                nc.vector.reciprocal(rden2[:], o2v[:, :, 16])
                nc.vector.tensor_mul(
                    logits_all[:, g, :, :], o2v[:, :, 0:16],
                    rden2[:].broadcast_to([128, NCH, 16]))

                # lsm partial: exp + pairwise sums for this graph
                ex = elup.tile([128, NCH, 16], F32, tag="ex")
                nc.scalar.activation(ex[:].rearrange("p a b -> p (a b)"),
                                     logits_all[:, g, :, :].rearrange(
                                         "p a b -> p (a b)"), AF.Exp)
                s8 = elup.tile([128, NCH, 8], F32, tag="s8")
                nc.vector.tensor_add(s8[:], ex[:, :, 0:8], ex[:, :, 8:16])
                s4 = elup.tile([128, NCH, 4], F32, tag="s4")
                nc.vector.tensor_add(s4[:], s8[:, :, 0:4], s8[:, :, 4:8])
                s2t = elup.tile([128, NCH, 2], F32, tag="s2t")
                nc.vector.tensor_add(s2t[:], s4[:, :, 0:2], s4[:, :, 2:4])
                nc.vector.tensor_add(s1_all[:, g, :], s2t[:, :, 0],
                                     s2t[:, :, 1])

                if dbg and g == dbg_g:
                    nc.sync.dma_start(out=dbg_lg.ap(),
                                      in_=logits_all[:, dbg_g, :, :])

            # ---------- deferred log_softmax (one Ln table switch) ------
            lse = finp.tile([128, GPC, NCH], F32)
            nc.scalar.activation(lse[:], s1_all[:], AF.Ln)
            fin = finp.tile([128, GPC, NCH, 16], F32)
            nc.vector.tensor_sub(
                fin[:], logits_all[:],
                lse[:].broadcast_to([128, GPC, NCH, 16]))
            nc.sync.dma_start(
                out=out_d.ap().rearrange("g (c p) k -> p g c k", c=NCH),
                in_=fin[:])
    return nc


def host_prep(adj, vertices, local_emb, emb0, emb1, w1, a_src1, a_dst1,
              w2, a_src2, a_dst2):
    """Build the 8 per-core input maps from full inputs."""
    adj = np.asarray(adj, dtype=np.float32)
    vertices = np.asarray(vertices)
    local_emb = np.asarray(local_emb, dtype=np.float32)
    emb0 = np.asarray(emb0, dtype=np.float32)
    emb1 = np.asarray(emb1, dtype=np.float32)
    w1 = np.asarray(w1, dtype=np.float32)
    a_src1 = np.asarray(a_src1, dtype=np.float32)
    a_dst1 = np.asarray(a_dst1, dtype=np.float32)
    w2 = np.asarray(w2, dtype=np.float32)
    a_src2 = np.asarray(a_src2, dtype=np.float32)
    a_dst2 = np.asarray(a_dst2, dtype=np.float32)

    hp_np = BFNP if HP_DT == mybir.dt.bfloat16 else np.float32
    vtx = vertices.astype(np.int64)
    # h: [b, n, 160] -> hT [b, 160, n]
    h = np.concatenate([emb0[vtx], emb1[vtx], local_emb], axis=2)
    hT = np.ascontiguousarray(h.transpose(0, 2, 1)).astype(hp_np)

    # adjT packed: [b, 128, 4*512] bf16, block jc = adjT rows jc*128..
    adjT = adj.transpose(0, 2, 1)
    adjTp = np.ascontiguousarray(
        adjT.reshape(BS, NCH, 128, N).transpose(0, 2, 1, 3).reshape(
            BS, 128, NCH * N)).astype(BFNP)

    w1f = np.ascontiguousarray(
        w1.transpose(1, 0, 2).reshape(FIN1, 512)).astype(hp_np)
    asrc = np.zeros((512, NH), np.float32)
    adst = np.zeros((512, NH), np.float32)
    for hh in range(NH):
        asrc[hh * 64:(hh + 1) * 64, hh] = a_src1[hh, :, 0]
        adst[hh * 64:(hh + 1) * 64, hh] = a_dst1[hh, :, 0]
    consts = {
        "w1f": w1f,
        "asrc": asrc.astype(BFNP),
        "adst": adst.astype(BFNP),
        "w2f": w2[0].astype(BFNP),
        "negcs": (-w2[0].sum(axis=0, keepdims=True)).astype(BFNP),
        "a2s": a_src2[0].astype(BFNP),
        "a2d": a_dst2[0].astype(BFNP),
        "ident": np.eye(128, dtype=np.float32).astype(BFNP),
        "bc8": np.repeat(np.eye(NH, dtype=np.float32), 128,
                         axis=1).astype(BFNP),
    }
    in_maps = []
    for core in range(NCORES):
        sl = slice(core * GPC, (core + 1) * GPC)
        m = dict(consts)
        m["hT4"] = np.ascontiguousarray(hT[sl])
        m["adjTp"] = np.ascontiguousarray(adjTp[sl])
        in_maps.append(m)
    return in_maps


_NC_CACHE = {}


def _get_nc():
    if "nc" not in _NC_CACHE:
        nc = build_nc()
        nc.compile()
        _NC_CACHE["nc"] = nc
    return _NC_CACHE["nc"]


def kernel(**inputs):
    from concourse.bass_utils import run_bass_kernel_spmd
    nc = _get_nc()
    in_maps = host_prep(**inputs)
    res = run_bass_kernel_spmd(nc, in_maps, core_ids=list(range(NCORES)))
    out = np.concatenate([r["out"] for r in res.results], axis=0)
    return out.astype(np.float32)


if __name__ == "__main__":
    nc = build_nc()
    print("built ok")


# revision 50
# speedup vs baseline: 1.0370x; 1.0370x over previous
"""TRN2 Bass kernel for nn_BatchDenseGAT (2-layer dense GAT, bs=32, n=512).

Sharding: data-parallel over the 32 graphs -> 4 graphs per NeuronCore x 8
cores, params replicated. Host does embedding gather/concat/transpose and
mask packing; all model math runs on device.

Device strategy (rank-1 attention factorization): for a GAT layer,
  exp(lrelu(s_i + d_j)) * adj[j,i]  ==  v_i * adjT[j,i] * p_j * max(w_j, r_i)
with u=exp(d), p=exp(0.2 d), w=exp(0.8 d), r=exp(-0.8 s), v=exp(s).
The v_i factor is constant along the softmax axis (j) and cancels against
the denominator, so the masked unnormalized weights are
  em'[j,i] = adjT[j,i] * p_j * max(w_j, r_i)
which needs only O(n) exponentials (rows/cols) instead of O(n^2): the n^2
work reduces to one 4x-mode tensor_scalar (max,mult) + one bf16
tensor_tensor (mask) per head, replacing the Prelu+Exp activation passes.

Aggregation runs "flipped" (out[i,o+den] via lhsT=em' blocks, ones-col in
the rhs for the denominator) so the softmax denominator is a per-partition
scalar: normalize rides an ACT copy's per-partition `scale`, elu is
  h2 = exp(min(v,0)) + relu(v) - 1,   v = out1 * rden
with 4x-mode dual-op tensor_scalars, and h2 is transposed back to [f, i]
with PE transposes for layer 2. The "-1" folds into layer-2 weights
(negcs). r broadcasts [128,512] come from one matmul with a constant
one-hot selector lhsT (bc8). Layer 2 repeats the scheme with one head.
log_softmax: per-graph Exp + pairwise-sum partials, one deferred Ln at the
end (single act-table switch; everything else lives in `exp_and_others`),
one batched output DMA.

Engine balance (cost-model tuned): the n^2 ops are split per head across
DVE/Pool (em' mask-mul 3:1 split), ACT takes broadcasts/copies/exps, PE
takes matmuls/transposes. fp32r is avoided (schedule-dependent corruption
on the NEFF path); h' matmuls run in bf16.
"""
import os
import sys
import numpy as np

sys.path.insert(0, '/opt/trn_rl_repo')

import ml_dtypes  # noqa: E402
import concourse.bacc as bacc  # noqa: E402
import concourse.bass as bass  # noqa: E402
import concourse.tile as tile  # noqa: E402
from concourse import mybir  # noqa: E402

F32 = mybir.dt.float32
F32R = mybir.dt.float32r
BF16 = mybir.dt.bfloat16
AF = mybir.ActivationFunctionType
ALU = mybir.AluOpType
BFNP = ml_dtypes.bfloat16

BS, N, NH, FO = 32, 512, 8, 64
HP_DT = {"f32r": mybir.dt.float32r, "f32": mybir.dt.float32,
         "bf16": mybir.dt.bfloat16}[os.environ.get("GAT_HP_DT", "bf16")]
FIN1 = 160
NCORES = 8
GPC = BS // NCORES  # graphs per core
NCH = 4             # 512 / 128 partition chunks

# engine knobs: per-head choices, tuned against TimelineSim.
# (gpsimd/Pool cannot read PSUM, so PSUM->SBUF copies are ACT/DVE only)
# em' mask-multiply engine per head: 'v' = DVE, 'g' = Pool/gpsimd
EM_ENG = os.environ.get("GAT_EM_ENG", "66666666")
# elu path per head: 'a' = ACT copy-scale + 4x DVE min/max,
#                    'v' = DVE dual-op TS direct from PSUM
ELU_ENG = os.environ.get("GAT_ELU_ENG", "aaaaaaaa")
# elu add engine per head: 'v' = DVE, 'g' = Pool
AD_ENG = os.environ.get("GAT_AD_ENG", "vvvvvvvv")
# r_bc copy engine per head: 'a' = ACT, 'v' = DVE
RB_ENG = os.environ.get("GAT_RB_ENG", "aaaaaaaa")
# h2cT copy engine per head: 'v' = DVE, 'a' = ACT
H2_ENG = os.environ.get("GAT_H2_ENG", "aaaavvvv")
# hpx chunk copy engine per ic: 'a' = ACT, 'v' = DVE
HPX_ENG = os.environ.get("GAT_HPX_ENG", "vvvv")
# c-chunk offload per head: '-' = all DVE, 'g' = last chunk on Pool,
# 'G' = last two chunks on Pool
CH_ENG = os.environ.get("GAT_CH_ENG", "gggggggg")
# elu relu-part (p_t) engine per head: 'v' = DVE, 'g' = Pool (off-chain)
MPP_ENG = os.environ.get("GAT_MPP_ENG", "vvvvvvvv")


def build_nc():
    B = 1 if os.environ.get("GAT_SERIAL", "0") == "1" else None
    dbg = os.environ.get("GAT_DEBUG_DUMP", "0") == "1"
    dbg_lvl = int(os.environ.get("GAT_DEBUG_LVL", "3"))
    dbg_g = int(os.environ.get("GAT_DEBUG_G", "0"))
    nc = bacc.Bacc("TRN2", target_bir_lowering=False, debug=False)

    hT4 = nc.dram_tensor("hT4", [GPC, FIN1, N], HP_DT, kind="ExternalInput")
    adjTp = nc.dram_tensor("adjTp", [GPC, 128, NCH * N], BF16,
                           kind="ExternalInput")
    w1f_d = nc.dram_tensor("w1f", [FIN1, 512], HP_DT, kind="ExternalInput")
    asrc_d = nc.dram_tensor("asrc", [512, NH], BF16, kind="ExternalInput")
    adst_d = nc.dram_tensor("adst", [512, NH], BF16, kind="ExternalInput")
    w2f_d = nc.dram_tensor("w2f", [512, 16], BF16, kind="ExternalInput")
    negcs_d = nc.dram_tensor("negcs", [1, 16], BF16, kind="ExternalInput")
    a2s_d = nc.dram_tensor("a2s", [16, 1], BF16, kind="ExternalInput")
    a2d_d = nc.dram_tensor("a2d", [16, 1], BF16, kind="ExternalInput")
    ident_d = nc.dram_tensor("ident", [128, 128], BF16, kind="ExternalInput")
    bc8_d = nc.dram_tensor("bc8", [NH, NH * 128], BF16, kind="ExternalInput")
    out_d = nc.dram_tensor("out", [GPC, N, 16], F32, kind="ExternalOutput")
    if dbg:
        dbg_tT = nc.dram_tensor("dbg_tT", [128, NCH, 512], BF16,
                                kind="ExternalOutput")
        dbg_rT = nc.dram_tensor("dbg_rT", [1, NH * 512], BF16,
                                kind="ExternalOutput")
        dbg_wp = nc.dram_tensor("dbg_wp", [128, 2, NCH, NH], F32,
                                kind="ExternalOutput")
        dbg_em = nc.dram_tensor("dbg_em", [128, NCH * 512], BF16,
                                kind="ExternalOutput")
        dbg_h2v = nc.dram_tensor("dbg_h2v", [128, NCH, 64], BF16,
                                 kind="ExternalOutput")
        dbg_h2cT = nc.dram_tensor("dbg_h2cT", [128, NCH, 512], BF16,
                                  kind="ExternalOutput")
        dbg_rden = nc.dram_tensor("dbg_rden", [128, NCH], F32,
                                  kind="ExternalOutput")
        dbg_lg = nc.dram_tensor("dbg_lg", [128, NCH, 16], F32,
                                kind="ExternalOutput")

    with tile.TileContext(nc) as tc:
        with tc.tile_pool(name="consts", bufs=1) as consts, \
             tc.tile_pool(name="gbuf", bufs=(B or int(os.environ.get("GAT_GBUF", "3")))) as gbuf, \
             tc.tile_pool(name="attn", bufs=(B or int(os.environ.get("GAT_ATTN", "3")))) as attn, \
             tc.tile_pool(name="small", bufs=(B or 2)) as small, \
             tc.tile_pool(name="elu", bufs=(B or int(os.environ.get("GAT_ELUB", "3")))) as elup, \
             tc.tile_pool(name="fin", bufs=1) as finp, \
             tc.tile_pool(name="ps_h", bufs=(B or int(os.environ.get("GAT_PSH", "2"))), space="PSUM") as ps_h, \
             tc.tile_pool(name="ps_rb", bufs=(B or 1), space="PSUM") as ps_rb, \
             tc.tile_pool(name="ps_sT", bufs=(B or 1), space="PSUM") as ps_sT, \
             tc.tile_pool(name="ps_row", bufs=(B or 1), space="PSUM") as ps_row, \
             tc.tile_pool(name="ps_o", bufs=(B or int(os.environ.get("GAT_PSO", "2"))), space="PSUM") as ps_o, \
             tc.tile_pool(name="ps_t", bufs=(B or 1), space="PSUM") as ps_t:

            # ---------- constants ----------
            w1f_a = consts.tile([128, 512], HP_DT)
            w1f_b = consts.tile([32, 512], HP_DT)
            nc.sync.dma_start(out=w1f_a, in_=w1f_d.ap()[0:128, :])
            nc.sync.dma_start(out=w1f_b, in_=w1f_d.ap()[128:160, :])

            # prefetch graph-0 loads ahead of the small consts: HWDGE is
            # serial (~625ns/DMA) and these gate the first matmuls.
            g_order = list(range(GPC))
            if os.environ.get("GAT_REV", "0") == "1":
                g_order = g_order[::-1]
            g0 = g_order[0]
            pref_hT_a = gbuf.tile([128, N], HP_DT, tag="hT_a")
            pref_hT_b = gbuf.tile([32, N], HP_DT, tag="hT_b")
            pref_adjT = gbuf.tile([128, NCH * N], BF16, tag="adjT")
            pref = {"hT_a": pref_hT_a, "hT_b": pref_hT_b, "adjT": pref_adjT}
            nc.sync.dma_start(out=pref_hT_a, in_=hT4.ap()[g0, 0:128, :])
            nc.sync.dma_start(out=pref_hT_b, in_=hT4.ap()[g0, 128:160, :])
            nc.sync.dma_start(out=pref_adjT, in_=adjTp.ap()[g0])

            asrc_sb = consts.tile([128, NCH, NH], BF16)
            adst_sb = consts.tile([128, NCH, NH], BF16)
            nc.sync.dma_start(out=asrc_sb,
                              in_=asrc_d.ap().rearrange("(c p) a -> p c a", c=NCH))
            nc.sync.dma_start(out=adst_sb,
                              in_=adst_d.ap().rearrange("(c p) a -> p c a", c=NCH))
            w2f_sb = consts.tile([128, NCH, 16], BF16)
            nc.sync.dma_start(out=w2f_sb,
                              in_=w2f_d.ap().rearrange("(c p) a -> p c a", c=NCH))
            negcs_sb = consts.tile([1, 16], BF16)
            nc.sync.dma_start(out=negcs_sb, in_=negcs_d.ap())
            a2s_sb = consts.tile([16, 1], BF16)
            a2d_sb = consts.tile([16, 1], BF16)
            nc.sync.dma_start(out=a2s_sb, in_=a2s_d.ap())
            nc.sync.dma_start(out=a2d_sb, in_=a2d_d.ap())
            ident = consts.tile([128, 128], BF16)
            nc.sync.dma_start(out=ident, in_=ident_d.ap())
            bc8 = consts.tile([NH, NH * 128], BF16)
            nc.sync.dma_start(out=bc8, in_=bc8_d.ap())
            onesrow = consts.tile([1, 512], BF16)
            nc.vector.memset(onesrow, 1.0)

            logits_all = finp.tile([128, GPC, NCH, 16], F32)
            s1_all = finp.tile([128, GPC, NCH], F32)

            for g in g_order:
                # ---------- graph loads ----------
                if g == g0:
                    hT_a, hT_b, adjT_sb = (pref["hT_a"], pref["hT_b"],
                                           pref["adjT"])
                else:
                    hT_a = gbuf.tile([128, N], HP_DT, tag="hT_a")
                    hT_b = gbuf.tile([32, N], HP_DT, tag="hT_b")
                    nc.sync.dma_start(out=hT_a, in_=hT4.ap()[g, 0:128, :])
                    nc.sync.dma_start(out=hT_b, in_=hT4.ap()[g, 128:160, :])
                    adjT_sb = gbuf.tile([128, NCH * N], BF16, tag="adjT")
                    nc.sync.dma_start(out=adjT_sb, in_=adjTp.ap()[g])

                # ---------- h_primeT (o-major) -> tanh -> tT bf16 ----------
                # (first: its tanh -> sT -> rT chain gates the heads)
                tT = gbuf.tile([128, NCH, 512], BF16, tag="tT")
                for oc in range(NCH):
                    hpT_ps = ps_h.tile([128, 512], F32, tag="big")
                    nc.tensor.matmul(hpT_ps[:],
                                     w1f_a[:, oc * 128:(oc + 1) * 128],
                                     hT_a[:],
                                     start=True, stop=False)
                    nc.tensor.matmul(hpT_ps[:],
                                     w1f_b[:, oc * 128:(oc + 1) * 128],
                                     hT_b[:],
                                     start=False, stop=True)
                    nc.scalar.activation(tT[:, oc, :], hpT_ps[:], AF.Tanh)

                # ---------- sT row [8, 512] -> rT = exp(-0.8 s) ----------
                sT_ps = ps_sT.tile([16, 512], F32, tag="sT")
                for oc in range(NCH):
                    nc.tensor.matmul(sT_ps[0:NH, :], asrc_sb[:, oc, :],
                                     tT[:, oc, :],
                                     start=(oc == 0), stop=(oc == NCH - 1))
                rT = small.tile([NH, 512], BF16, tag="rT")
                nc.scalar.activation(rT[:], sT_ps[0:NH, :], AF.Exp, scale=-0.8)

                # -- h_prime (n-major) -> hpx bf16 [j, (jc), (h, 64+ones)] --
                hpx = gbuf.tile([128, NCH, NH, 65], BF16, tag="hpx")
                nc.vector.memset(hpx[:, :, :, 64:65], 1.0)
                for ic in range(NCH):
                    hp_ps = ps_h.tile([128, 512], F32, tag="big")
                    nc.tensor.matmul(hp_ps[:],
                                     hT_a[:, ic * 128:(ic + 1) * 128],
                                     w1f_a[:],
                                     start=True, stop=False)
                    nc.tensor.matmul(hp_ps[:],
                                     hT_b[:, ic * 128:(ic + 1) * 128],
                                     w1f_b[:],
                                     start=False, stop=True)
                    if HPX_ENG[ic] == 'a':
                        nc.scalar.copy(
                            hpx[:, ic, :, 0:64],
                            hp_ps[:].rearrange("p (h o) -> p h o", h=NH))
                    else:
                        nc.vector.tensor_copy(
                            hpx[:, ic, :, 0:64],
                            hp_ps[:].rearrange("p (h o) -> p h o", h=NH))


                # ---------- d cols -> w = exp(0.8 d), p = exp(0.2 d) ------
                d_ps = ps_o.tile([128, NCH, 65], F32, tag="o65")
                for jc in range(NCH):
                    for oc in range(NCH):
                        nc.tensor.matmul(d_ps[:, jc, 0:NH],
                                         tT[:, oc, jc * 128:(jc + 1) * 128],
                                         adst_sb[:, oc, :],
                                         start=(oc == 0), stop=(oc == NCH - 1))
                w_sb = small.tile([128, NCH, NH], F32, tag="w_sb")
                p_sb = small.tile([128, NCH, NH], F32, tag="p_sb")
                nc.scalar.activation(w_sb[:], d_ps[:, :, 0:NH], AF.Exp,
                                     scale=0.8)
                nc.scalar.activation(p_sb[:], d_ps[:, :, 0:NH], AF.Exp,
                                     scale=0.2)

                if dbg and g == dbg_g and dbg_lvl >= 2:
                    nc.sync.dma_start(out=dbg_tT.ap(), in_=tT[:])
                    nc.sync.dma_start(out=dbg_rT.ap(), in_=rT1[:])
                    nc.sync.dma_start(out=dbg_wp.ap()[:, 0], in_=w_sb[:])
                    nc.sync.dma_start(out=dbg_wp.ap()[:, 1], in_=p_sb[:])

                # ---------- per-head attention ----------
                h2cT = gbuf.tile([128, NCH, 512], BF16, tag="h2cT")
                for h in range(NH):
                    # r broadcast [128, 512] via PE rank-1, copy to bf16 sbuf
                    rbps = ps_rb.tile([128, 512], F32, tag="rb")
                    nc.tensor.matmul(rbps[:],
                                     bc8[:, h * 128:(h + 1) * 128],
                                     rT[:], start=True, stop=True)
                    r_bc = attn.tile([128, 512], BF16, tag="r_bc")
                    if RB_ENG[h] == 'a':
                        nc.scalar.copy(r_bc[:], rbps[:])
                    else:
                        nc.vector.tensor_copy(r_bc[:], rbps[:])

                    # c[j,i] = max(w_j, r_i) * p_j  (4x-mode dual-op TS)
                    c_all = attn.tile([128, NCH, 512], BF16, tag="c_all")
                    n_pool_c = {'-': 0, 'g': 1, 'G': 2}[CH_ENG[h]]
                    for jc in range(NCH):
                        eng = (nc.gpsimd if jc >= NCH - n_pool_c
                               else nc.vector)
                        eng.tensor_scalar(
                            c_all[:, jc, :], r_bc[:],
                            w_sb[:, jc, h:h + 1], p_sb[:, jc, h:h + 1],
                            ALU.max, ALU.mult)
                    # em' = c * adjT (split across DVE/Pool to balance)
                    em_all = attn.tile([128, NCH * 512], BF16, tag="em")
                    cflat = c_all[:].rearrange("p a b -> p (a b)")
                    ch = EM_ENG[h]
                    if ch == 'g':
                        nc.gpsimd.tensor_mul(em_all[:], cflat, adjT_sb[:])
                    elif ch == 'v':
                        nc.vector.tensor_mul(em_all[:], cflat, adjT_sb[:])
                    else:
                        cut = int(ch) * 256
                        nc.vector.tensor_mul(em_all[:, 0:cut],
                                             cflat[:, 0:cut],
                                             adjT_sb[:, 0:cut])
                        nc.gpsimd.tensor_mul(em_all[:, cut:2048],
                                             cflat[:, cut:2048],
                                             adjT_sb[:, cut:2048])

                    # out1 flipped: [i, 64+den] per ic chunk
                    o65v = ps_o.tile([128, NCH, 65], F32, tag="o65")
                    for ic in range(NCH):
                        for jc in range(NCH):
                            nc.tensor.matmul(
                                o65v[:, ic, :],
                                em_all[:, jc * 512 + ic * 128:
                                       jc * 512 + (ic + 1) * 128],
                                hpx[:, jc, h, :],
                                start=(jc == 0), stop=(jc == NCH - 1))
                    rden = elup.tile([128, NCH], F32, tag="rden")
                    nc.vector.reciprocal(rden[:], o65v[:, :, 64])

                    # elu: m = min(v,0), p = relu(v), v = out1*rden
                    m_t = elup.tile([128, NCH, 64], BF16, tag="m_t")
                    p_t = elup.tile([128, NCH, 64], BF16, tag="p_t")
                    if ELU_ENG[h] in 'abd':
                        # fold the per-partition rden into a scaled copy;
                        # min/max are then 4x-mode SBUF tensor_scalars.
                        # 'b': one DVE tensor_tensor with a stride-0
                        # broadcast view of rden (cheapest op count).
                        v_sb = elup.tile([128, NCH, 64], BF16, tag="v_sb")
                        if ELU_ENG[h] == 'b':
                            nc.vector.tensor_mul(
                                v_sb[:], o65v[:, :, 0:64],
                                rden[:].broadcast_to([128, NCH, 64]))
                        else:
                            for ic in range(NCH):
                                if ELU_ENG[h] == 'a':
                                    nc.scalar.activation(
                                        v_sb[:, ic, :], o65v[:, ic, 0:64],
                                        AF.Copy, scale=rden[:, ic:ic + 1])
                                else:
                                    nc.vector.tensor_scalar(
                                        v_sb[:, ic, :], o65v[:, ic, 0:64],
                                        rden[:, ic:ic + 1], None, ALU.mult)
                        vf = v_sb[:].rearrange("p a b -> p (a b)")
                        nc.vector.tensor_scalar(
                            m_t[:].rearrange("p a b -> p (a b)"), vf,
                            0.0, None, ALU.min)
                        peng = (nc.gpsimd if MPP_ENG[h] == 'g'
                                else nc.vector)
                        peng.tensor_scalar(
                            p_t[:].rearrange("p a b -> p (a b)"), vf,
                            0.0, None, ALU.max)
                    else:
                        for ic in range(NCH):
                            nc.vector.tensor_scalar(
                                m_t[:, ic, :], o65v[:, ic, 0:64],
                                rden[:, ic:ic + 1], 0.0, ALU.mult, ALU.min)
                            nc.vector.tensor_scalar(
                                p_t[:, ic, :], o65v[:, ic, 0:64],
                                rden[:, ic:ic + 1], 0.0, ALU.mult, ALU.max)
                    em_t = elup.tile([128, NCH, 64], BF16, tag="em_t")
                    nc.scalar.activation(
                        em_t[:].rearrange("p a b -> p (a b)"),
                        m_t[:].rearrange("p a b -> p (a b)"), AF.Exp)
                    h2v = elup.tile([128, NCH, 64], BF16, tag="h2v")
                    if AD_ENG[h] == 'g':
                        nc.gpsimd.tensor_add(
                            h2v[:].rearrange("p a b -> p (a b)"),
                            em_t[:].rearrange("p a b -> p (a b)"),
                            p_t[:].rearrange("p a b -> p (a b)"))
                    else:
                        nc.vector.tensor_add(
                            h2v[:].rearrange("p a b -> p (a b)"),
                            em_t[:].rearrange("p a b -> p (a b)"),
                            p_t[:].rearrange("p a b -> p (a b)"))

                    if dbg and g == dbg_g and h == 0 and dbg_lvl >= 3:
                        nc.sync.dma_start(out=dbg_em.ap(), in_=em_all[:])
                        nc.sync.dma_start(out=dbg_h2v.ap(), in_=h2v[:])
                        nc.sync.dma_start(out=dbg_rden.ap(), in_=rden[:])

                    # transpose h2v [i,(ic),64] -> h2cT rows [64, 512]
                    h2T_ps = ps_t.tile([64, 512], BF16, tag="t")
                    for ic in range(NCH):
                        nc.tensor.transpose(
                            h2T_ps[:, ic * 128:(ic + 1) * 128],
                            h2v[:, ic, :], ident[:])
                    prow = (h % 2) * 64
                    if H2_ENG[h] == 'a':
                        nc.scalar.copy(h2cT[prow:prow + 64, h // 2, :],
                                       h2T_ps[:])
                    else:
                        nc.vector.tensor_copy(
                            h2cT[prow:prow + 64, h // 2, :], h2T_ps[:])

                if dbg and g == dbg_g:
                    nc.sync.dma_start(out=dbg_h2cT.ap(), in_=h2cT[:])

                # ================= layer 2 =================
                # h_prime2 (n-major) [i, 16] + ones col -> hp2x bf16
                hp2_ps = ps_o.tile([128, NCH, 65], F32, tag="o65")
                for ic in range(NCH):
                    for fc in range(NCH):
                        nc.tensor.matmul(hp2_ps[:, ic, 0:16],
                                         h2cT[:, fc, ic * 128:(ic + 1) * 128],
                                         w2f_sb[:, fc, :],
                                         start=(fc == 0), stop=False)
                    nc.tensor.matmul(hp2_ps[:, ic, 0:16],
                                     onesrow[:, ic * 128:(ic + 1) * 128],
                                     negcs_sb[:],
                                     start=False, stop=True)
                hp2x = small.tile([128, NCH, 17], BF16, tag="hp2x")
                nc.vector.tensor_copy(hp2x[:, :, 0:16], hp2_ps[:, :, 0:16])
                nc.vector.memset(hp2x[:, :, 16:17], 1.0)

                # h_prime2T [16, n] -> tanh -> t2 bf16
                hp2T_ps = ps_row.tile([16, 512], F32, tag="row")
                for fc in range(NCH):
                    nc.tensor.matmul(hp2T_ps[:], w2f_sb[:, fc, :],
                                     h2cT[:, fc, :],
                                     start=(fc == 0), stop=False)
                nc.tensor.matmul(hp2T_ps[:], negcs_sb[:], onesrow[:],
                                 start=False, stop=True)
                t2_sb = small.tile([16, 512], BF16, tag="t2")
                nc.scalar.activation(t2_sb[:], hp2T_ps[:], AF.Tanh)

                # s2 row -> r2 = exp(-0.8 s2); d2 cols -> w2c, p2c
                s2_ps = ps_row.tile([16, 512], F32, tag="row")
                nc.tensor.matmul(s2_ps[0:1, :], a2s_sb[:], t2_sb[:],
                                 start=True, stop=True)
                r2 = small.tile([1, 512], BF16, tag="r2")
                nc.scalar.activation(r2[:], s2_ps[0:1, :], AF.Exp, scale=-0.8)
                d2_ps = ps_o.tile([128, NCH, 65], F32, tag="o65")
                for jc in range(NCH):
                    nc.tensor.matmul(d2_ps[:, jc, 0:1],
                                     t2_sb[:, jc * 128:(jc + 1) * 128],
                                     a2d_sb[:], start=True, stop=True)
                w2c = small.tile([128, NCH], F32, tag="w2c")
                p2c = small.tile([128, NCH], F32, tag="p2c")
                nc.scalar.activation(w2c[:], d2_ps[:, :, 0], AF.Exp, scale=0.8)
                nc.scalar.activation(p2c[:], d2_ps[:, :, 0], AF.Exp, scale=0.2)

                # r2 broadcast + c2 + em2
                rb2ps = ps_rb.tile([128, 512], F32, tag="rb")
                nc.tensor.matmul(rb2ps[:], onesrow[:, 0:128], r2[:],
                                 start=True, stop=True)
                r2_bc = attn.tile([128, 512], BF16, tag="r_bc")
                nc.scalar.copy(r2_bc[:], rb2ps[:])
                c2_all = attn.tile([128, NCH, 512], BF16, tag="c_all")
                for jc in range(NCH):
                    nc.vector.tensor_scalar(
                        c2_all[:, jc, :], r2_bc[:],
                        w2c[:, jc:jc + 1], p2c[:, jc:jc + 1],
                        ALU.max, ALU.mult)
                em2_all = attn.tile([128, NCH * 512], BF16, tag="em")
                c2flat = c2_all[:].rearrange("p a b -> p (a b)")
                nc.vector.tensor_mul(em2_all[:, 0:1536], c2flat[:, 0:1536],
                                     adjT_sb[:, 0:1536])
                nc.gpsimd.tensor_mul(em2_all[:, 1536:2048],
                                     c2flat[:, 1536:2048],
                                     adjT_sb[:, 1536:2048])

                # out2 flipped [i, 16+den] per ic; logits = out2 * rden2
                o2v = ps_o.tile([128, NCH, 65], F32, tag="o65")
                for ic in range(NCH):
                    for jc in range(NCH):
                        nc.tensor.matmul(
                            o2v[:, ic, 0:17],
                            em2_all[:, jc * 512 + ic * 128:
                                    jc * 512 + (ic + 1) * 128],
                            hp2x[:, jc, :],
                            start=(jc == 0), stop=(jc == NCH - 1))
                rden2 = elup.tile([128, NCH], F32, tag="rden")
                nc.vector.reciprocal(rden2[:], o2v[:, :, 16])
                nc.vector.tensor_mul(
                    logits_all[:, g, :, :], o2v[:, :, 0:16],
                    rden2[:].broadcast_to([128, NCH, 16]))

                # lsm partial: exp + pairwise sums for this graph
                ex = elup.tile([128, NCH, 16], F32, tag="ex")
                nc.scalar.activation(ex[:].rearrange("p a b -> p (a b)"),
                                     logits_all[:, g, :, :].rearrange(
                                         "p a b -> p (a b)"), AF.Exp)
                s8 = elup.tile([128, NCH, 8], F32, tag="s8")
                nc.vector.tensor_add(s8[:], ex[:, :, 0:8], ex[:, :, 8:16])
                s4 = elup.tile([128, NCH, 4], F32, tag="s4")
                nc.vector.tensor_add(s4[:], s8[:, :, 0:4], s8[:, :, 4:8])
                s2t = elup.tile([128, NCH, 2], F32, tag="s2t")
                nc.vector.tensor_add(s2t[:], s4[:, :, 0:2], s4[:, :, 2:4])
                nc.vector.tensor_add(s1_all[:, g, :], s2t[:, :, 0],
                                     s2t[:, :, 1])

                if dbg and g == dbg_g:
                    nc.sync.dma_start(out=dbg_lg.ap(),
                                      in_=logits_all[:, dbg_g, :, :])

            # ---------- deferred log_softmax (one Ln table switch) ------
            lse = finp.tile([128, GPC, NCH], F32)
            nc.scalar.activation(lse[:], s1_all[:], AF.Ln)
            fin = finp.tile([128, GPC, NCH, 16], F32)
            nc.vector.tensor_sub(
                fin[:], logits_all[:],
                lse[:].broadcast_to([128, GPC, NCH, 16]))
            nc.sync.dma_start(
                out=out_d.ap().rearrange("g (c p) k -> p g c k", c=NCH),
                in_=fin[:])
    return nc


def host_prep(adj, vertices, local_emb, emb0, emb1, w1, a_src1, a_dst1,
              w2, a_src2, a_dst2):
    """Build the 8 per-core input maps from full inputs."""
    adj = np.asarray(adj, dtype=np.float32)
    vertices = np.asarray(vertices)
    local_emb = np.asarray(local_emb, dtype=np.float32)
    emb0 = np.asarray(emb0, dtype=np.float32)
    emb1 = np.asarray(emb1, dtype=np.float32)
    w1 = np.asarray(w1, dtype=np.float32)
    a_src1 = np.asarray(a_src1, dtype=np.float32)
    a_dst1 = np.asarray(a_dst1, dtype=np.float32)
    w2 = np.asarray(w2, dtype=np.float32)
    a_src2 = np.asarray(a_src2, dtype=np.float32)
    a_dst2 = np.asarray(a_dst2, dtype=np.float32)

    hp_np = BFNP if HP_DT == mybir.dt.bfloat16 else np.float32
    vtx = vertices.astype(np.int64)
    # h: [b, n, 160] -> hT [b, 160, n]
    h = np.concatenate([emb0[vtx], emb1[vtx], local_emb], axis=2)
    hT = np.ascontiguousarray(h.transpose(0, 2, 1)).astype(hp_np)

    # adjT packed: [b, 128, 4*512] bf16, block jc = adjT rows jc*128..
    adjT = adj.transpose(0, 2, 1)
    adjTp = np.ascontiguousarray(
        adjT.reshape(BS, NCH, 128, N).transpose(0, 2, 1, 3).reshape(
            BS, 128, NCH * N)).astype(BFNP)

    w1f = np.ascontiguousarray(
        w1.transpose(1, 0, 2).reshape(FIN1, 512)).astype(hp_np)
    asrc = np.zeros((512, NH), np.float32)
    adst = np.zeros((512, NH), np.float32)
    for hh in range(NH):
        asrc[hh * 64:(hh + 1) * 64, hh] = a_src1[hh, :, 0]
        adst[hh * 64:(hh + 1) * 64, hh] = a_dst1[hh, :, 0]
    consts = {
        "w1f": w1f,
        "asrc": asrc.astype(BFNP),
        "adst": adst.astype(BFNP),
        "w2f": w2[0].astype(BFNP),
        "negcs": (-w2[0].sum(axis=0, keepdims=True)).astype(BFNP),
        "a2s": a_src2[0].astype(BFNP),
        "a2d": a_dst2[0].astype(BFNP),
        "ident": np.eye(128, dtype=np.float32).astype(BFNP),
        "bc8": np.repeat(np.eye(NH, dtype=np.float32), 128,
                         axis=1).astype(BFNP),
    }
    in_maps = []
    for core in range(NCORES):
        sl = slice(core * GPC, (core + 1) * GPC)
        m = dict(consts)
        m["hT4"] = np.ascontiguousarray(hT[sl])
        m["adjTp"] = np.ascontiguousarray(adjTp[sl])
        in_maps.append(m)
    return in_maps


_NC_CACHE = {}


def _get_nc():
    if "nc" not in _NC_CACHE:
        nc = build_nc()
        nc.compile()
        _NC_CACHE["nc"] = nc
    return _NC_CACHE["nc"]


def kernel(**inputs):
    from concourse.bass_utils import run_bass_kernel_spmd
    nc = _get_nc()
    in_maps = host_prep(**inputs)
    res = run_bass_kernel_spmd(nc, in_maps, core_ids=list(range(NCORES)))
    out = np.concatenate([r["out"] for r in res.results], axis=0)
    return out.astype(np.float32)


if __name__ == "__main__":
    nc = build_nc()
    print("built ok")


# revision 51
# speedup vs baseline: 1.0543x; 1.0166x over previous
"""TRN2 Bass kernel for nn_BatchDenseGAT (2-layer dense GAT, bs=32, n=512).

Sharding: data-parallel over the 32 graphs -> 4 graphs per NeuronCore x 8
cores, params replicated. Host does embedding gather/concat/transpose and
mask packing; all model math runs on device.

Device strategy (rank-1 attention factorization): for a GAT layer,
  exp(lrelu(s_i + d_j)) * adj[j,i]  ==  v_i * adjT[j,i] * p_j * max(w_j, r_i)
with u=exp(d), p=exp(0.2 d), w=exp(0.8 d), r=exp(-0.8 s), v=exp(s).
The v_i factor is constant along the softmax axis (j) and cancels against
the denominator, so the masked unnormalized weights are
  em'[j,i] = adjT[j,i] * p_j * max(w_j, r_i)
which needs only O(n) exponentials (rows/cols) instead of O(n^2): the n^2
work reduces to one 4x-mode tensor_scalar (max,mult) + one bf16
tensor_tensor (mask) per head, replacing the Prelu+Exp activation passes.

Aggregation runs "flipped" (out[i,o+den] via lhsT=em' blocks, ones-col in
the rhs for the denominator) so the softmax denominator is a per-partition
scalar: normalize rides an ACT copy's per-partition `scale`, elu is
  h2 = exp(min(v,0)) + relu(v) - 1,   v = out1 * rden
with 4x-mode dual-op tensor_scalars, and h2 is transposed back to [f, i]
with PE transposes for layer 2. The "-1" folds into layer-2 weights
(negcs). r broadcasts [128,512] come from one matmul with a constant
one-hot selector lhsT (bc8). Layer 2 repeats the scheme with one head.
log_softmax: per-graph Exp + pairwise-sum partials, one deferred Ln at the
end (single act-table switch; everything else lives in `exp_and_others`),
one batched output DMA.

Engine balance (cost-model tuned): the n^2 ops are split per head across
DVE/Pool (em' mask-mul 3:1 split), ACT takes broadcasts/copies/exps, PE
takes matmuls/transposes. fp32r is avoided (schedule-dependent corruption
on the NEFF path); h' matmuls run in bf16.
"""
import os
import sys
import numpy as np

sys.path.insert(0, '/opt/trn_rl_repo')

import ml_dtypes  # noqa: E402
import concourse.bacc as bacc  # noqa: E402
import concourse.bass as bass  # noqa: E402
import concourse.tile as tile  # noqa: E402
from concourse import mybir  # noqa: E402

F32 = mybir.dt.float32
F32R = mybir.dt.float32r
BF16 = mybir.dt.bfloat16
AF = mybir.ActivationFunctionType
ALU = mybir.AluOpType
BFNP = ml_dtypes.bfloat16

BS, N, NH, FO = 32, 512, 8, 64
HP_DT = {"f32r": mybir.dt.float32r, "f32": mybir.dt.float32,
         "bf16": mybir.dt.bfloat16}[os.environ.get("GAT_HP_DT", "bf16")]
FIN1 = 160
NCORES = 8
GPC = BS // NCORES  # graphs per core
NCH = 4             # 512 / 128 partition chunks

# engine knobs: per-head choices, tuned against TimelineSim.
# (gpsimd/Pool cannot read PSUM, so PSUM->SBUF copies are ACT/DVE only)
# em' mask-multiply engine per head: 'v' = DVE, 'g' = Pool/gpsimd
EM_ENG = os.environ.get("GAT_EM_ENG", "66666666")
# elu path per head: 'a' = ACT copy-scale + 4x DVE min/max,
#                    'v' = DVE dual-op TS direct from PSUM
ELU_ENG = os.environ.get("GAT_ELU_ENG", "baaaaaab")
# elu add engine per head: 'v' = DVE, 'g' = Pool
AD_ENG = os.environ.get("GAT_AD_ENG", "vvvvvvvv")
# r_bc copy engine per head: 'a' = ACT, 'v' = DVE
RB_ENG = os.environ.get("GAT_RB_ENG", "aaaaaaaa")
# h2cT copy engine per head: 'v' = DVE, 'a' = ACT
H2_ENG = os.environ.get("GAT_H2_ENG", "aaaavvvv")
# hpx chunk copy engine per ic: 'a' = ACT, 'v' = DVE
HPX_ENG = os.environ.get("GAT_HPX_ENG", "vvvv")
# c-chunk offload per head: '-' = all DVE, 'g' = last chunk on Pool,
# 'G' = last two chunks on Pool
CH_ENG = os.environ.get("GAT_CH_ENG", "gggggggg")
# elu relu-part (p_t) engine per head: 'v' = DVE, 'g' = Pool (off-chain)
MPP_ENG = os.environ.get("GAT_MPP_ENG", "vvvvvvvv")


def build_nc():
    B = 1 if os.environ.get("GAT_SERIAL", "0") == "1" else None
    dbg = os.environ.get("GAT_DEBUG_DUMP", "0") == "1"
    dbg_lvl = int(os.environ.get("GAT_DEBUG_LVL", "3"))
    dbg_g = int(os.environ.get("GAT_DEBUG_G", "0"))
    nc = bacc.Bacc("TRN2", target_bir_lowering=False, debug=False)

    hT4 = nc.dram_tensor("hT4", [GPC, FIN1, N], HP_DT, kind="ExternalInput")
    adjTp = nc.dram_tensor("adjTp", [GPC, 128, NCH * N], BF16,
                           kind="ExternalInput")
    w1f_d = nc.dram_tensor("w1f", [FIN1, 512], HP_DT, kind="ExternalInput")
    asrc_d = nc.dram_tensor("asrc", [512, NH], BF16, kind="ExternalInput")
    adst_d = nc.dram_tensor("adst", [512, NH], BF16, kind="ExternalInput")
    w2f_d = nc.dram_tensor("w2f", [512, 16], BF16, kind="ExternalInput")
    negcs_d = nc.dram_tensor("negcs", [1, 16], BF16, kind="ExternalInput")
    a2s_d = nc.dram_tensor("a2s", [16, 1], BF16, kind="ExternalInput")
    a2d_d = nc.dram_tensor("a2d", [16, 1], BF16, kind="ExternalInput")
    ident_d = nc.dram_tensor("ident", [128, 128], BF16, kind="ExternalInput")
    bc8_d = nc.dram_tensor("bc8", [NH, NH * 128], BF16, kind="ExternalInput")
    out_d = nc.dram_tensor("out", [GPC, N, 16], F32, kind="ExternalOutput")
    if dbg:
        dbg_tT = nc.dram_tensor("dbg_tT", [128, NCH, 512], BF16,
                                kind="ExternalOutput")
        dbg_rT = nc.dram_tensor("dbg_rT", [1, NH * 512], BF16,
                                kind="ExternalOutput")
        dbg_wp = nc.dram_tensor("dbg_wp", [128, 2, NCH, NH], F32,
                                kind="ExternalOutput")
        dbg_em = nc.dram_tensor("dbg_em", [128, NCH * 512], BF16,
                                kind="ExternalOutput")
        dbg_h2v = nc.dram_tensor("dbg_h2v", [128, NCH, 64], BF16,
                                 kind="ExternalOutput")
        dbg_h2cT = nc.dram_tensor("dbg_h2cT", [128, NCH, 512], BF16,
                                  kind="ExternalOutput")
        dbg_rden = nc.dram_tensor("dbg_rden", [128, NCH], F32,
                                  kind="ExternalOutput")
        dbg_lg = nc.dram_tensor("dbg_lg", [128, NCH, 16], F32,
                                kind="ExternalOutput")

    with tile.TileContext(nc) as tc:
        with tc.tile_pool(name="consts", bufs=1) as consts, \
             tc.tile_pool(name="gbuf", bufs=(B or int(os.environ.get("GAT_GBUF", "3")))) as gbuf, \
             tc.tile_pool(name="attn", bufs=(B or int(os.environ.get("GAT_ATTN", "3")))) as attn, \
             tc.tile_pool(name="small", bufs=(B or 2)) as small, \
             tc.tile_pool(name="elu", bufs=(B or int(os.environ.get("GAT_ELUB", "3")))) as elup, \
             tc.tile_pool(name="fin", bufs=1) as finp, \
             tc.tile_pool(name="ps_h", bufs=(B or int(os.environ.get("GAT_PSH", "2"))), space="PSUM") as ps_h, \
             tc.tile_pool(name="ps_rb", bufs=(B or 1), space="PSUM") as ps_rb, \
             tc.tile_pool(name="ps_sT", bufs=(B or 1), space="PSUM") as ps_sT, \
             tc.tile_pool(name="ps_row", bufs=(B or 1), space="PSUM") as ps_row, \
             tc.tile_pool(name="ps_o", bufs=(B or int(os.environ.get("GAT_PSO", "2"))), space="PSUM") as ps_o, \
             tc.tile_pool(name="ps_t", bufs=(B or 1), space="PSUM") as ps_t:

            # ---------- constants ----------
            w1f_a = consts.tile([128, 512], HP_DT)
            w1f_b = consts.tile([32, 512], HP_DT)
            nc.sync.dma_start(out=w1f_a, in_=w1f_d.ap()[0:128, :])
            nc.sync.dma_start(out=w1f_b, in_=w1f_d.ap()[128:160, :])

            # prefetch graph-0 loads ahead of the small consts: HWDGE is
            # serial (~625ns/DMA) and these gate the first matmuls.
            g_order = list(range(GPC))
            if os.environ.get("GAT_REV", "0") == "1":
                g_order = g_order[::-1]
            g0 = g_order[0]
            pref_hT_a = gbuf.tile([128, N], HP_DT, tag="hT_a")
            pref_hT_b = gbuf.tile([32, N], HP_DT, tag="hT_b")
            pref_adjT = gbuf.tile([128, NCH * N], BF16, tag="adjT")
            pref = {"hT_a": pref_hT_a, "hT_b": pref_hT_b, "adjT": pref_adjT}
            nc.sync.dma_start(out=pref_hT_a, in_=hT4.ap()[g0, 0:128, :])
            nc.sync.dma_start(out=pref_hT_b, in_=hT4.ap()[g0, 128:160, :])
            nc.sync.dma_start(out=pref_adjT, in_=adjTp.ap()[g0])

            asrc_sb = consts.tile([128, NCH, NH], BF16)
            adst_sb = consts.tile([128, NCH, NH], BF16)
            nc.sync.dma_start(out=asrc_sb,
                              in_=asrc_d.ap().rearrange("(c p) a -> p c a", c=NCH))
            nc.sync.dma_start(out=adst_sb,
                              in_=adst_d.ap().rearrange("(c p) a -> p c a", c=NCH))
            w2f_sb = consts.tile([128, NCH, 16], BF16)
            nc.sync.dma_start(out=w2f_sb,
                              in_=w2f_d.ap().rearrange("(c p) a -> p c a", c=NCH))
            negcs_sb = consts.tile([1, 16], BF16)
            nc.sync.dma_start(out=negcs_sb, in_=negcs_d.ap())
            a2s_sb = consts.tile([16, 1], BF16)
            a2d_sb = consts.tile([16, 1], BF16)
            nc.sync.dma_start(out=a2s_sb, in_=a2s_d.ap())
            nc.sync.dma_start(out=a2d_sb, in_=a2d_d.ap())
            ident = consts.tile([128, 128], BF16)
            nc.sync.dma_start(out=ident, in_=ident_d.ap())
            bc8 = consts.tile([NH, NH * 128], BF16)
            nc.sync.dma_start(out=bc8, in_=bc8_d.ap())
            onesrow = consts.tile([1, 512], BF16)
            nc.vector.memset(onesrow, 1.0)

            logits_all = finp.tile([128, GPC, NCH, 16], F32)
            s1_all = finp.tile([128, GPC, NCH], F32)

            for g in g_order:
                # ---------- graph loads ----------
                if g == g0:
                    hT_a, hT_b, adjT_sb = (pref["hT_a"], pref["hT_b"],
                                           pref["adjT"])
                else:
                    hT_a = gbuf.tile([128, N], HP_DT, tag="hT_a")
                    hT_b = gbuf.tile([32, N], HP_DT, tag="hT_b")
                    nc.sync.dma_start(out=hT_a, in_=hT4.ap()[g, 0:128, :])
                    nc.sync.dma_start(out=hT_b, in_=hT4.ap()[g, 128:160, :])
                    adjT_sb = gbuf.tile([128, NCH * N], BF16, tag="adjT")
                    nc.sync.dma_start(out=adjT_sb, in_=adjTp.ap()[g])

                # ---------- h_primeT (o-major) -> tanh -> tT bf16 ----------
                # (first: its tanh -> sT -> rT chain gates the heads)
                tT = gbuf.tile([128, NCH, 512], BF16, tag="tT")
                for oc in range(NCH):
                    hpT_ps = ps_h.tile([128, 512], F32, tag="big")
                    nc.tensor.matmul(hpT_ps[:],
                                     w1f_a[:, oc * 128:(oc + 1) * 128],
                                     hT_a[:],
                                     start=True, stop=False)
                    nc.tensor.matmul(hpT_ps[:],
                                     w1f_b[:, oc * 128:(oc + 1) * 128],
                                     hT_b[:],
                                     start=False, stop=True)
                    nc.scalar.activation(tT[:, oc, :], hpT_ps[:], AF.Tanh)

                # ---------- sT row [8, 512] -> rT = exp(-0.8 s) ----------
                sT_ps = ps_sT.tile([16, 512], F32, tag="sT")
                for oc in range(NCH):
                    nc.tensor.matmul(sT_ps[0:NH, :], asrc_sb[:, oc, :],
                                     tT[:, oc, :],
                                     start=(oc == 0), stop=(oc == NCH - 1))
                rT = small.tile([NH, 512], BF16, tag="rT")
                nc.scalar.activation(rT[:], sT_ps[0:NH, :], AF.Exp, scale=-0.8)

                # -- h_prime (n-major) -> hpx bf16 [j, (jc), (h, 64+ones)] --
                hpx = gbuf.tile([128, NCH, NH, 65], BF16, tag="hpx")
                nc.vector.memset(hpx[:, :, :, 64:65], 1.0)
                for ic in range(NCH):
                    hp_ps = ps_h.tile([128, 512], F32, tag="big")
                    nc.tensor.matmul(hp_ps[:],
                                     hT_a[:, ic * 128:(ic + 1) * 128],
                                     w1f_a[:],
                                     start=True, stop=False)
                    nc.tensor.matmul(hp_ps[:],
                                     hT_b[:, ic * 128:(ic + 1) * 128],
                                     w1f_b[:],
                                     start=False, stop=True)
                    if HPX_ENG[ic] == 'a':
                        nc.scalar.copy(
                            hpx[:, ic, :, 0:64],
                            hp_ps[:].rearrange("p (h o) -> p h o", h=NH))
                    else:
                        nc.vector.tensor_copy(
                            hpx[:, ic, :, 0:64],
                            hp_ps[:].rearrange("p (h o) -> p h o", h=NH))


                # ---------- d cols -> w = exp(0.8 d), p = exp(0.2 d) ------
                d_ps = ps_o.tile([128, NCH, 65], F32, tag="o65")
                for jc in range(NCH):
                    for oc in range(NCH):
                        nc.tensor.matmul(d_ps[:, jc, 0:NH],
                                         tT[:, oc, jc * 128:(jc + 1) * 128],
                                         adst_sb[:, oc, :],
                                         start=(oc == 0), stop=(oc == NCH - 1))
                w_sb = small.tile([128, NCH, NH], F32, tag="w_sb")
                p_sb = small.tile([128, NCH, NH], F32, tag="p_sb")
                nc.scalar.activation(w_sb[:], d_ps[:, :, 0:NH], AF.Exp,
                                     scale=0.8)
                nc.scalar.activation(p_sb[:], d_ps[:, :, 0:NH], AF.Exp,
                                     scale=0.2)

                if dbg and g == dbg_g and dbg_lvl >= 2:
                    nc.sync.dma_start(out=dbg_tT.ap(), in_=tT[:])
                    nc.sync.dma_start(out=dbg_rT.ap(), in_=rT1[:])
                    nc.sync.dma_start(out=dbg_wp.ap()[:, 0], in_=w_sb[:])
                    nc.sync.dma_start(out=dbg_wp.ap()[:, 1], in_=p_sb[:])

                # ---------- per-head attention ----------
                h2cT = gbuf.tile([128, NCH, 512], BF16, tag="h2cT")
                for h in range(NH):
                    # r broadcast [128, 512] via PE rank-1, copy to bf16 sbuf
                    rbps = ps_rb.tile([128, 512], F32, tag="rb")
                    nc.tensor.matmul(rbps[:],
                                     bc8[:, h * 128:(h + 1) * 128],
                                     rT[:], start=True, stop=True)
                    r_bc = attn.tile([128, 512], BF16, tag="r_bc")
                    if RB_ENG[h] == 'a':
                        nc.scalar.copy(r_bc[:], rbps[:])
                    else:
                        nc.vector.tensor_copy(r_bc[:], rbps[:])

                    # c[j,i] = max(w_j, r_i) * p_j  (4x-mode dual-op TS)
                    c_all = attn.tile([128, NCH, 512], BF16, tag="c_all")
                    n_pool_c = {'-': 0, 'g': 1, 'G': 2}[CH_ENG[h]]
                    for jc in range(NCH):
                        eng = (nc.gpsimd if jc >= NCH - n_pool_c
                               else nc.vector)
                        eng.tensor_scalar(
                            c_all[:, jc, :], r_bc[:],
                            w_sb[:, jc, h:h + 1], p_sb[:, jc, h:h + 1],
                            ALU.max, ALU.mult)
                    # em' = c * adjT (split across DVE/Pool to balance)
                    em_all = attn.tile([128, NCH * 512], BF16, tag="em")
                    cflat = c_all[:].rearrange("p a b -> p (a b)")
                    ch = EM_ENG[h]
                    if ch == 'g':
                        nc.gpsimd.tensor_mul(em_all[:], cflat, adjT_sb[:])
                    elif ch == 'v':
                        nc.vector.tensor_mul(em_all[:], cflat, adjT_sb[:])
                    else:
                        cut = int(ch) * 256
                        nc.vector.tensor_mul(em_all[:, 0:cut],
                                             cflat[:, 0:cut],
                                             adjT_sb[:, 0:cut])
                        nc.gpsimd.tensor_mul(em_all[:, cut:2048],
                                             cflat[:, cut:2048],
                                             adjT_sb[:, cut:2048])

                    # out1 flipped: [i, 64+den] per ic chunk
                    o65v = ps_o.tile([128, NCH, 65], F32, tag="o65")
                    for ic in range(NCH):
                        for jc in range(NCH):
                            nc.tensor.matmul(
                                o65v[:, ic, :],
                                em_all[:, jc * 512 + ic * 128:
                                       jc * 512 + (ic + 1) * 128],
                                hpx[:, jc, h, :],
                                start=(jc == 0), stop=(jc == NCH - 1))
                    rden = elup.tile([128, NCH], F32, tag="rden")
                    nc.vector.reciprocal(rden[:], o65v[:, :, 64])

                    # elu: m = min(v,0), p = relu(v), v = out1*rden
                    m_t = elup.tile([128, NCH, 64], BF16, tag="m_t")
                    p_t = elup.tile([128, NCH, 64], BF16, tag="p_t")
                    if ELU_ENG[h] in 'abd':
                        # fold the per-partition rden into a scaled copy;
                        # min/max are then 4x-mode SBUF tensor_scalars.
                        # 'b': one DVE tensor_tensor with a stride-0
                        # broadcast view of rden (cheapest op count).
                        v_sb = elup.tile([128, NCH, 64], BF16, tag="v_sb")
                        if ELU_ENG[h] == 'b':
                            nc.vector.tensor_mul(
                                v_sb[:], o65v[:, :, 0:64],
                                rden[:].broadcast_to([128, NCH, 64]))
                        else:
                            for ic in range(NCH):
                                if ELU_ENG[h] == 'a':
                                    nc.scalar.activation(
                                        v_sb[:, ic, :], o65v[:, ic, 0:64],
                                        AF.Copy, scale=rden[:, ic:ic + 1])
                                else:
                                    nc.vector.tensor_scalar(
                                        v_sb[:, ic, :], o65v[:, ic, 0:64],
                                        rden[:, ic:ic + 1], None, ALU.mult)
                        vf = v_sb[:].rearrange("p a b -> p (a b)")
                        nc.vector.tensor_scalar(
                            m_t[:].rearrange("p a b -> p (a b)"), vf,
                            0.0, None, ALU.min)
                        peng = (nc.gpsimd if MPP_ENG[h] == 'g'
                                else nc.vector)
                        peng.tensor_scalar(
                            p_t[:].rearrange("p a b -> p (a b)"), vf,
                            0.0, None, ALU.max)
                    else:
                        for ic in range(NCH):
                            nc.vector.tensor_scalar(
                                m_t[:, ic, :], o65v[:, ic, 0:64],
                                rden[:, ic:ic + 1], 0.0, ALU.mult, ALU.min)
                            nc.vector.tensor_scalar(
                                p_t[:, ic, :], o65v[:, ic, 0:64],
                                rden[:, ic:ic + 1], 0.0, ALU.mult, ALU.max)
                    em_t = elup.tile([128, NCH, 64], BF16, tag="em_t")
                    nc.scalar.activation(
                        em_t[:].rearrange("p a b -> p (a b)"),
                        m_t[:].rearrange("p a b -> p (a b)"), AF.Exp)
                    h2v = elup.tile([128, NCH, 64], BF16, tag="h2v")
                    if AD_ENG[h] == 'g':
                        nc.gpsimd.tensor_add(
                            h2v[:].rearrange("p a b -> p (a b)"),
                            em_t[:].rearrange("p a b -> p (a b)"),
                            p_t[:].rearrange("p a b -> p (a b)"))
                    else:
                        nc.vector.tensor_add(
                            h2v[:].rearrange("p a b -> p (a b)"),
                            em_t[:].rearrange("p a b -> p (a b)"),
                            p_t[:].rearrange("p a b -> p (a b)"))

                    if dbg and g == dbg_g and h == 0 and dbg_lvl >= 3:
                        nc.sync.dma_start(out=dbg_em.ap(), in_=em_all[:])
                        nc.sync.dma_start(out=dbg_h2v.ap(), in_=h2v[:])
                        nc.sync.dma_start(out=dbg_rden.ap(), in_=rden[:])

                    # transpose h2v [i,(ic),64] -> h2cT rows [64, 512]
                    h2T_ps = ps_t.tile([64, 512], BF16, tag="t")
                    for ic in range(NCH):
                        nc.tensor.transpose(
                            h2T_ps[:, ic * 128:(ic + 1) * 128],
                            h2v[:, ic, :], ident[:])
                    prow = (h % 2) * 64
                    if H2_ENG[h] == 'a':
                        nc.scalar.copy(h2cT[prow:prow + 64, h // 2, :],
                                       h2T_ps[:])
                    else:
                        nc.vector.tensor_copy(
                            h2cT[prow:prow + 64, h // 2, :], h2T_ps[:])

                if dbg and g == dbg_g:
                    nc.sync.dma_start(out=dbg_h2cT.ap(), in_=h2cT[:])

                # ================= layer 2 =================
                # h_prime2 (n-major) [i, 16] + ones col -> hp2x bf16
                hp2_ps = ps_o.tile([128, NCH, 65], F32, tag="o65")
                for ic in range(NCH):
                    for fc in range(NCH):
                        nc.tensor.matmul(hp2_ps[:, ic, 0:16],
                                         h2cT[:, fc, ic * 128:(ic + 1) * 128],
                                         w2f_sb[:, fc, :],
                                         start=(fc == 0), stop=False)
                    nc.tensor.matmul(hp2_ps[:, ic, 0:16],
                                     onesrow[:, ic * 128:(ic + 1) * 128],
                                     negcs_sb[:],
                                     start=False, stop=True)
                hp2x = small.tile([128, NCH, 17], BF16, tag="hp2x")
                nc.vector.tensor_copy(hp2x[:, :, 0:16], hp2_ps[:, :, 0:16])
                nc.vector.memset(hp2x[:, :, 16:17], 1.0)

                # h_prime2T [16, n] -> tanh -> t2 bf16
                hp2T_ps = ps_row.tile([16, 512], F32, tag="row")
                for fc in range(NCH):
                    nc.tensor.matmul(hp2T_ps[:], w2f_sb[:, fc, :],
                                     h2cT[:, fc, :],
                                     start=(fc == 0), stop=False)
                nc.tensor.matmul(hp2T_ps[:], negcs_sb[:], onesrow[:],
                                 start=False, stop=True)
                t2_sb = small.tile([16, 512], BF16, tag="t2")
                nc.scalar.activation(t2_sb[:], hp2T_ps[:], AF.Tanh)

                # s2 row -> r2 = exp(-0.8 s2); d2 cols -> w2c, p2c
                s2_ps = ps_row.tile([16, 512], F32, tag="row")
                nc.tensor.matmul(s2_ps[0:1, :], a2s_sb[:], t2_sb[:],
                                 start=True, stop=True)
                r2 = small.tile([1, 512], BF16, tag="r2")
                nc.scalar.activation(r2[:], s2_ps[0:1, :], AF.Exp, scale=-0.8)
                d2_ps = ps_o.tile([128, NCH, 65], F32, tag="o65")
                for jc in range(NCH):
                    nc.tensor.matmul(d2_ps[:, jc, 0:1],
                                     t2_sb[:, jc * 128:(jc + 1) * 128],
                                     a2d_sb[:], start=True, stop=True)
                w2c = small.tile([128, NCH], F32, tag="w2c")
                p2c = small.tile([128, NCH], F32, tag="p2c")
                nc.scalar.activation(w2c[:], d2_ps[:, :, 0], AF.Exp, scale=0.8)
                nc.scalar.activation(p2c[:], d2_ps[:, :, 0], AF.Exp, scale=0.2)

                # r2 broadcast + c2 + em2
                rb2ps = ps_rb.tile([128, 512], F32, tag="rb")
                nc.tensor.matmul(rb2ps[:], onesrow[:, 0:128], r2[:],
                                 start=True, stop=True)
                r2_bc = attn.tile([128, 512], BF16, tag="r_bc")
                nc.scalar.copy(r2_bc[:], rb2ps[:])
                c2_all = attn.tile([128, NCH, 512], BF16, tag="c_all")
                for jc in range(NCH):
                    nc.vector.tensor_scalar(
                        c2_all[:, jc, :], r2_bc[:],
                        w2c[:, jc:jc + 1], p2c[:, jc:jc + 1],
                        ALU.max, ALU.mult)
                em2_all = attn.tile([128, NCH * 512], BF16, tag="em")
                c2flat = c2_all[:].rearrange("p a b -> p (a b)")
                nc.vector.tensor_mul(em2_all[:, 0:1536], c2flat[:, 0:1536],
                                     adjT_sb[:, 0:1536])
                nc.gpsimd.tensor_mul(em2_all[:, 1536:2048],
                                     c2flat[:, 1536:2048],
                                     adjT_sb[:, 1536:2048])

                # out2 flipped [i, 16+den] per ic; logits = out2 * rden2
                o2v = ps_o.tile([128, NCH, 65], F32, tag="o65")
                for ic in range(NCH):
                    for jc in range(NCH):
                        nc.tensor.matmul(
                            o2v[:, ic, 0:17],
                            em2_all[:, jc * 512 + ic * 128:
                                    jc * 512 + (ic + 1) * 128],
                            hp2x[:, jc, :],
                            start=(jc == 0), stop=(jc == NCH - 1))
                rden2 = elup.tile([128, NCH], F32, tag="rden")
                nc.vector.reciprocal(rden2[:], o2v[:, :, 16])
                nc.vector.tensor_mul(
                    logits_all[:, g, :, :], o2v[:, :, 0:16],
                    rden2[:].broadcast_to([128, NCH, 16]))

                # lsm partial: exp + pairwise sums for this graph
                ex = elup.tile([128, NCH, 16], F32, tag="ex")
                nc.scalar.activation(ex[:].rearrange("p a b -> p (a b)"),
                                     logits_all[:, g, :, :].rearrange(
                                         "p a b -> p (a b)"), AF.Exp)
                s8 = elup.tile([128, NCH, 8], F32, tag="s8")
                nc.vector.tensor_add(s8[:], ex[:, :, 0:8], ex[:, :, 8:16])
                s4 = elup.tile([128, NCH, 4], F32, tag="s4")
                nc.vector.tensor_add(s4[:], s8[:, :, 0:4], s8[:, :, 4:8])
                s2t = elup.tile([128, NCH, 2], F32, tag="s2t")
                nc.vector.tensor_add(s2t[:], s4[:, :, 0:2], s4[:, :, 2:4])
                nc.vector.tensor_add(s1_all[:, g, :], s2t[:, :, 0],
                                     s2t[:, :, 1])

                if dbg and g == dbg_g:
                    nc.sync.dma_start(out=dbg_lg.ap(),
                                      in_=logits_all[:, dbg_g, :, :])

            # ---------- deferred log_softmax (one Ln table switch) ------
            lse = finp.tile([128, GPC, NCH], F32)
            nc.scalar.activation(lse[:], s1_all[:], AF.Ln)
            fin = finp.tile([128, GPC, NCH, 16], F32)
            nc.vector.tensor_sub(
                fin[:], logits_all[:],
                lse[:].broadcast_to([128, GPC, NCH, 16]))
            nc.sync.dma_start(
                out=out_d.ap().rearrange("g (c p) k -> p g c k", c=NCH),
                in_=fin[:])
    return nc


def host_prep(adj, vertices, local_emb, emb0, emb1, w1, a_src1, a_dst1,
              w2, a_src2, a_dst2):
    """Build the 8 per-core input maps from full inputs."""
    adj = np.asarray(adj, dtype=np.float32)
    vertices = np.asarray(vertices)
    local_emb = np.asarray(local_emb, dtype=np.float32)
    emb0 = np.asarray(emb0, dtype=np.float32)
    emb1 = np.asarray(emb1, dtype=np.float32)
    w1 = np.asarray(w1, dtype=np.float32)
    a_src1 = np.asarray(a_src1, dtype=np.float32)
    a_dst1 = np.asarray(a_dst1, dtype=np.float32)
    w2 = np.asarray(w2, dtype=np.float32)
    a_src2 = np.asarray(a_src2, dtype=np.float32)
    a_dst2 = np.asarray(a_dst2, dtype=np.float32)

    hp_np = BFNP if HP_DT == mybir.dt.bfloat16 else np.float32
    vtx = vertices.astype(np.int64)
    # h: [b, n, 160] -> hT [b, 160, n]
    h = np.concatenate([emb0[vtx], emb1[vtx], local_emb], axis=2)
    hT = np.ascontiguousarray(h.transpose(0, 2, 1)).astype(hp_np)

    # adjT packed: [b, 128, 4*512] bf16, block jc = adjT rows jc*128..
    adjT = adj.transpose(0, 2, 1)
    adjTp = np.ascontiguousarray(
        adjT.reshape(BS, NCH, 128, N).transpose(0, 2, 1, 3).reshape(
            BS, 128, NCH * N)).astype(BFNP)

    w1f = np.ascontiguousarray(
        w1.transpose(1, 0, 2).reshape(FIN1, 512)).astype(hp_np)
    asrc = np.zeros((512, NH), np.float32)
    adst = np.zeros((512, NH), np.float32)
    for hh in range(NH):
        asrc[hh * 64:(hh + 1) * 64, hh] = a_src1[hh, :, 0]
        adst[hh * 64:(hh + 1) * 64, hh] = a_dst1[hh, :, 0]
    consts = {
        "w1f": w1f,
        "asrc": asrc.astype(BFNP),
        "adst": adst.astype(BFNP),
        "w2f": w2[0].astype(BFNP),
        "negcs": (-w2[0].sum(axis=0, keepdims=True)).astype(BFNP),
        "a2s": a_src2[0].astype(BFNP),
        "a2d": a_dst2[0].astype(BFNP),
        "ident": np.eye(128, dtype=np.float32).astype(BFNP),
        "bc8": np.repeat(np.eye(NH, dtype=np.float32), 128,
                         axis=1).astype(BFNP),
    }
    in_maps = []
    for core in range(NCORES):
        sl = slice(core * GPC, (core + 1) * GPC)
        m = dict(consts)
        m["hT4"] = np.ascontiguousarray(hT[sl])
        m["adjTp"] = np.ascontiguousarray(adjTp[sl])
        in_maps.append(m)
    return in_maps


_NC_CACHE = {}


def _get_nc():
    if "nc" not in _NC_CACHE:
        nc = build_nc()
        nc.compile()
        _NC_CACHE["nc"] = nc
    return _NC_CACHE["nc"]


def kernel(**inputs):
    from concourse.bass_utils import run_bass_kernel_spmd
    nc = _get_nc()
    in_maps = host_prep(**inputs)
    res = run_bass_kernel_spmd(nc, in_maps, core_ids=list(range(NCORES)))
    out = np.concatenate([r["out"] for r in res.results], axis=0)
    return out.astype(np.float32)


if __name__ == "__main__":
    nc = build_nc()
    print("built ok")


# revision 52
# speedup vs baseline: 1.0656x; 1.0107x over previous
"""TRN2 Bass kernel for nn_BatchDenseGAT (2-layer dense GAT, bs=32, n=512).

Sharding: data-parallel over the 32 graphs -> 4 graphs per NeuronCore x 8
cores, params replicated. Host does embedding gather/concat/transpose and
mask packing; all model math runs on device.

Device strategy (rank-1 attention factorization): for a GAT layer,
  exp(lrelu(s_i + d_j)) * adj[j,i]  ==  v_i * adjT[j,i] * p_j * max(w_j, r_i)
with u=exp(d), p=exp(0.2 d), w=exp(0.8 d), r=exp(-0.8 s), v=exp(s).
The v_i factor is constant along the softmax axis (j) and cancels against
the denominator, so the masked unnormalized weights are
  em'[j,i] = adjT[j,i] * p_j * max(w_j, r_i)
which needs only O(n) exponentials (rows/cols) instead of O(n^2): the n^2
work reduces to one 4x-mode tensor_scalar (max,mult) + one bf16
tensor_tensor (mask) per head, replacing the Prelu+Exp activation passes.

Aggregation runs "flipped" (out[i,o+den] via lhsT=em' blocks, ones-col in
the rhs for the denominator) so the softmax denominator is a per-partition
scalar: normalize rides an ACT copy's per-partition `scale`, elu is
  h2 = exp(min(v,0)) + relu(v) - 1,   v = out1 * rden
with 4x-mode dual-op tensor_scalars, and h2 is transposed back to [f, i]
with PE transposes for layer 2. The "-1" folds into layer-2 weights
(negcs). r broadcasts [128,512] come from one matmul with a constant
one-hot selector lhsT (bc8). Layer 2 repeats the scheme with one head.
log_softmax: per-graph Exp + pairwise-sum partials, one deferred Ln at the
end (single act-table switch; everything else lives in `exp_and_others`),
one batched output DMA.

Engine balance (cost-model tuned): the n^2 ops are split per head across
DVE/Pool (em' mask-mul 3:1 split), ACT takes broadcasts/copies/exps, PE
takes matmuls/transposes. fp32r is avoided (schedule-dependent corruption
on the NEFF path); h' matmuls run in bf16.
"""
import os
import sys
import numpy as np

sys.path.insert(0, '/opt/trn_rl_repo')

import ml_dtypes  # noqa: E402
import concourse.bacc as bacc  # noqa: E402
import concourse.bass as bass  # noqa: E402
import concourse.tile as tile  # noqa: E402
from concourse import mybir  # noqa: E402

F32 = mybir.dt.float32
F32R = mybir.dt.float32r
BF16 = mybir.dt.bfloat16
AF = mybir.ActivationFunctionType
ALU = mybir.AluOpType
BFNP = ml_dtypes.bfloat16

BS, N, NH, FO = 32, 512, 8, 64
HP_DT = {"f32r": mybir.dt.float32r, "f32": mybir.dt.float32,
         "bf16": mybir.dt.bfloat16}[os.environ.get("GAT_HP_DT", "bf16")]
FIN1 = 160
NCORES = 8
GPC = BS // NCORES  # graphs per core
NCH = 4             # 512 / 128 partition chunks

# engine knobs: per-head choices, tuned against TimelineSim.
# (gpsimd/Pool cannot read PSUM, so PSUM->SBUF copies are ACT/DVE only)
# em' mask-multiply engine per head: 'v' = DVE, 'g' = Pool/gpsimd
EM_ENG = os.environ.get("GAT_EM_ENG", "66666666")
# elu path per head: 'a' = ACT copy-scale + 4x DVE min/max,
#                    'v' = DVE dual-op TS direct from PSUM
ELU_ENG = os.environ.get("GAT_ELU_ENG", "baabaaab")
# elu add engine per head: 'v' = DVE, 'g' = Pool
AD_ENG = os.environ.get("GAT_AD_ENG", "vvvvvvvv")
# r_bc copy engine per head: 'a' = ACT, 'v' = DVE
RB_ENG = os.environ.get("GAT_RB_ENG", "aaaaaaaa")
# h2cT copy engine per head: 'v' = DVE, 'a' = ACT
H2_ENG = os.environ.get("GAT_H2_ENG", "aaaavvvv")
# hpx chunk copy engine per ic: 'a' = ACT, 'v' = DVE
HPX_ENG = os.environ.get("GAT_HPX_ENG", "vvvv")
# c-chunk offload per head: '-' = all DVE, 'g' = last chunk on Pool,
# 'G' = last two chunks on Pool
CH_ENG = os.environ.get("GAT_CH_ENG", "gggggggg")
# elu relu-part (p_t) engine per head: 'v' = DVE, 'g' = Pool (off-chain)
MPP_ENG = os.environ.get("GAT_MPP_ENG", "vvvvvvvv")


def build_nc():
    B = 1 if os.environ.get("GAT_SERIAL", "0") == "1" else None
    dbg = os.environ.get("GAT_DEBUG_DUMP", "0") == "1"
    dbg_lvl = int(os.environ.get("GAT_DEBUG_LVL", "3"))
    dbg_g = int(os.environ.get("GAT_DEBUG_G", "0"))
    nc = bacc.Bacc("TRN2", target_bir_lowering=False, debug=False)

    hT4 = nc.dram_tensor("hT4", [GPC, FIN1, N], HP_DT, kind="ExternalInput")
    adjTp = nc.dram_tensor("adjTp", [GPC, 128, NCH * N], BF16,
                           kind="ExternalInput")
    w1f_d = nc.dram_tensor("w1f", [FIN1, 512], HP_DT, kind="ExternalInput")
    asrc_d = nc.dram_tensor("asrc", [512, NH], BF16, kind="ExternalInput")
    adst_d = nc.dram_tensor("adst", [512, NH], BF16, kind="ExternalInput")
    w2f_d = nc.dram_tensor("w2f", [512, 16], BF16, kind="ExternalInput")
    negcs_d = nc.dram_tensor("negcs", [1, 16], BF16, kind="ExternalInput")
    a2s_d = nc.dram_tensor("a2s", [16, 1], BF16, kind="ExternalInput")
    a2d_d = nc.dram_tensor("a2d", [16, 1], BF16, kind="ExternalInput")
    ident_d = nc.dram_tensor("ident", [128, 128], BF16, kind="ExternalInput")
    bc8_d = nc.dram_tensor("bc8", [NH, NH * 128], BF16, kind="ExternalInput")
    out_d = nc.dram_tensor("out", [GPC, N, 16], F32, kind="ExternalOutput")
    if dbg:
        dbg_tT = nc.dram_tensor("dbg_tT", [128, NCH, 512], BF16,
                                kind="ExternalOutput")
        dbg_rT = nc.dram_tensor("dbg_rT", [1, NH * 512], BF16,
                                kind="ExternalOutput")
        dbg_wp = nc.dram_tensor("dbg_wp", [128, 2, NCH, NH], F32,
                                kind="ExternalOutput")
        dbg_em = nc.dram_tensor("dbg_em", [128, NCH * 512], BF16,
                                kind="ExternalOutput")
        dbg_h2v = nc.dram_tensor("dbg_h2v", [128, NCH, 64], BF16,
                                 kind="ExternalOutput")
        dbg_h2cT = nc.dram_tensor("dbg_h2cT", [128, NCH, 512], BF16,
                                  kind="ExternalOutput")
        dbg_rden = nc.dram_tensor("dbg_rden", [128, NCH], F32,
                                  kind="ExternalOutput")
        dbg_lg = nc.dram_tensor("dbg_lg", [128, NCH, 16], F32,
                                kind="ExternalOutput")

    with tile.TileContext(nc) as tc:
        with tc.tile_pool(name="consts", bufs=1) as consts, \
             tc.tile_pool(name="gbuf", bufs=(B or int(os.environ.get("GAT_GBUF", "3")))) as gbuf, \
             tc.tile_pool(name="attn", bufs=(B or int(os.environ.get("GAT_ATTN", "3")))) as attn, \
             tc.tile_pool(name="small", bufs=(B or 2)) as small, \
             tc.tile_pool(name="elu", bufs=(B or int(os.environ.get("GAT_ELUB", "3")))) as elup, \
             tc.tile_pool(name="fin", bufs=1) as finp, \
             tc.tile_pool(name="ps_h", bufs=(B or int(os.environ.get("GAT_PSH", "2"))), space="PSUM") as ps_h, \
             tc.tile_pool(name="ps_rb", bufs=(B or 1), space="PSUM") as ps_rb, \
             tc.tile_pool(name="ps_sT", bufs=(B or 1), space="PSUM") as ps_sT, \
             tc.tile_pool(name="ps_row", bufs=(B or 1), space="PSUM") as ps_row, \
             tc.tile_pool(name="ps_o", bufs=(B or int(os.environ.get("GAT_PSO", "2"))), space="PSUM") as ps_o, \
             tc.tile_pool(name="ps_t", bufs=(B or 1), space="PSUM") as ps_t:

            # ---------- constants ----------
            w1f_a = consts.tile([128, 512], HP_DT)
            w1f_b = consts.tile([32, 512], HP_DT)
            nc.sync.dma_start(out=w1f_a, in_=w1f_d.ap()[0:128, :])
            nc.sync.dma_start(out=w1f_b, in_=w1f_d.ap()[128:160, :])

            # prefetch graph-0 loads ahead of the small consts: HWDGE is
            # serial (~625ns/DMA) and these gate the first matmuls.
            g_order = list(range(GPC))
            if os.environ.get("GAT_REV", "0") == "1":
                g_order = g_order[::-1]
            g0 = g_order[0]
            pref_hT_a = gbuf.tile([128, N], HP_DT, tag="hT_a")
            pref_hT_b = gbuf.tile([32, N], HP_DT, tag="hT_b")
            pref_adjT = gbuf.tile([128, NCH * N], BF16, tag="adjT")
            pref = {"hT_a": pref_hT_a, "hT_b": pref_hT_b, "adjT": pref_adjT}
            nc.sync.dma_start(out=pref_hT_a, in_=hT4.ap()[g0, 0:128, :])
            nc.sync.dma_start(out=pref_hT_b, in_=hT4.ap()[g0, 128:160, :])
            nc.sync.dma_start(out=pref_adjT, in_=adjTp.ap()[g0])

            asrc_sb = consts.tile([128, NCH, NH], BF16)
            adst_sb = consts.tile([128, NCH, NH], BF16)
            nc.sync.dma_start(out=asrc_sb,
                              in_=asrc_d.ap().rearrange("(c p) a -> p c a", c=NCH))
            nc.sync.dma_start(out=adst_sb,
                              in_=adst_d.ap().rearrange("(c p) a -> p c a", c=NCH))
            w2f_sb = consts.tile([128, NCH, 16], BF16)
            nc.sync.dma_start(out=w2f_sb,
                              in_=w2f_d.ap().rearrange("(c p) a -> p c a", c=NCH))
            negcs_sb = consts.tile([1, 16], BF16)
            nc.sync.dma_start(out=negcs_sb, in_=negcs_d.ap())
            a2s_sb = consts.tile([16, 1], BF16)
            a2d_sb = consts.tile([16, 1], BF16)
            nc.sync.dma_start(out=a2s_sb, in_=a2s_d.ap())
            nc.sync.dma_start(out=a2d_sb, in_=a2d_d.ap())
            ident = consts.tile([128, 128], BF16)
            nc.sync.dma_start(out=ident, in_=ident_d.ap())
            bc8 = consts.tile([NH, NH * 128], BF16)
            nc.sync.dma_start(out=bc8, in_=bc8_d.ap())
            onesrow = consts.tile([1, 512], BF16)
            nc.vector.memset(onesrow, 1.0)

            logits_all = finp.tile([128, GPC, NCH, 16], F32)
            s1_all = finp.tile([128, GPC, NCH], F32)

            for g in g_order:
                # ---------- graph loads ----------
                if g == g0:
                    hT_a, hT_b, adjT_sb = (pref["hT_a"], pref["hT_b"],
                                           pref["adjT"])
                else:
                    hT_a = gbuf.tile([128, N], HP_DT, tag="hT_a")
                    hT_b = gbuf.tile([32, N], HP_DT, tag="hT_b")
                    nc.sync.dma_start(out=hT_a, in_=hT4.ap()[g, 0:128, :])
                    nc.sync.dma_start(out=hT_b, in_=hT4.ap()[g, 128:160, :])
                    adjT_sb = gbuf.tile([128, NCH * N], BF16, tag="adjT")
                    nc.sync.dma_start(out=adjT_sb, in_=adjTp.ap()[g])

                # ---------- h_primeT (o-major) -> tanh -> tT bf16 ----------
                # (first: its tanh -> sT -> rT chain gates the heads)
                tT = gbuf.tile([128, NCH, 512], BF16, tag="tT")
                for oc in range(NCH):
                    hpT_ps = ps_h.tile([128, 512], F32, tag="big")
                    nc.tensor.matmul(hpT_ps[:],
                                     w1f_a[:, oc * 128:(oc + 1) * 128],
                                     hT_a[:],
                                     start=True, stop=False)
                    nc.tensor.matmul(hpT_ps[:],
                                     w1f_b[:, oc * 128:(oc + 1) * 128],
                                     hT_b[:],
                                     start=False, stop=True)
                    nc.scalar.activation(tT[:, oc, :], hpT_ps[:], AF.Tanh)

                # ---------- sT row [8, 512] -> rT = exp(-0.8 s) ----------
                sT_ps = ps_sT.tile([16, 512], F32, tag="sT")
                for oc in range(NCH):
                    nc.tensor.matmul(sT_ps[0:NH, :], asrc_sb[:, oc, :],
                                     tT[:, oc, :],
                                     start=(oc == 0), stop=(oc == NCH - 1))
                rT = small.tile([NH, 512], BF16, tag="rT")
                nc.scalar.activation(rT[:], sT_ps[0:NH, :], AF.Exp, scale=-0.8)

                # -- h_prime (n-major) -> hpx bf16 [j, (jc), (h, 64+ones)] --
                hpx = gbuf.tile([128, NCH, NH, 65], BF16, tag="hpx")
                nc.vector.memset(hpx[:, :, :, 64:65], 1.0)
                for ic in range(NCH):
                    hp_ps = ps_h.tile([128, 512], F32, tag="big")
                    nc.tensor.matmul(hp_ps[:],
                                     hT_a[:, ic * 128:(ic + 1) * 128],
                                     w1f_a[:],
                                     start=True, stop=False)
                    nc.tensor.matmul(hp_ps[:],
                                     hT_b[:, ic * 128:(ic + 1) * 128],
                                     w1f_b[:],
                                     start=False, stop=True)
                    if HPX_ENG[ic] == 'a':
                        nc.scalar.copy(
                            hpx[:, ic, :, 0:64],
                            hp_ps[:].rearrange("p (h o) -> p h o", h=NH))
                    else:
                        nc.vector.tensor_copy(
                            hpx[:, ic, :, 0:64],
                            hp_ps[:].rearrange("p (h o) -> p h o", h=NH))


                # ---------- d cols -> w = exp(0.8 d), p = exp(0.2 d) ------
                d_ps = ps_o.tile([128, NCH, 65], F32, tag="o65")
                for jc in range(NCH):
                    for oc in range(NCH):
                        nc.tensor.matmul(d_ps[:, jc, 0:NH],
                                         tT[:, oc, jc * 128:(jc + 1) * 128],
                                         adst_sb[:, oc, :],
                                         start=(oc == 0), stop=(oc == NCH - 1))
                w_sb = small.tile([128, NCH, NH], F32, tag="w_sb")
                p_sb = small.tile([128, NCH, NH], F32, tag="p_sb")
                nc.scalar.activation(w_sb[:], d_ps[:, :, 0:NH], AF.Exp,
                                     scale=0.8)
                nc.scalar.activation(p_sb[:], d_ps[:, :, 0:NH], AF.Exp,
                                     scale=0.2)

                if dbg and g == dbg_g and dbg_lvl >= 2:
                    nc.sync.dma_start(out=dbg_tT.ap(), in_=tT[:])
                    nc.sync.dma_start(out=dbg_rT.ap(), in_=rT1[:])
                    nc.sync.dma_start(out=dbg_wp.ap()[:, 0], in_=w_sb[:])
                    nc.sync.dma_start(out=dbg_wp.ap()[:, 1], in_=p_sb[:])

                # ---------- per-head attention ----------
                h2cT = gbuf.tile([128, NCH, 512], BF16, tag="h2cT")
                for h in range(NH):
                    # r broadcast [128, 512] via PE rank-1, copy to bf16 sbuf
                    rbps = ps_rb.tile([128, 512], F32, tag="rb")
                    nc.tensor.matmul(rbps[:],
                                     bc8[:, h * 128:(h + 1) * 128],
                                     rT[:], start=True, stop=True)
                    r_bc = attn.tile([128, 512], BF16, tag="r_bc")
                    if RB_ENG[h] == 'a':
                        nc.scalar.copy(r_bc[:], rbps[:])
                    else:
                        nc.vector.tensor_copy(r_bc[:], rbps[:])

                    # c[j,i] = max(w_j, r_i) * p_j  (4x-mode dual-op TS)
                    c_all = attn.tile([128, NCH, 512], BF16, tag="c_all")
                    n_pool_c = {'-': 0, 'g': 1, 'G': 2}[CH_ENG[h]]
                    for jc in range(NCH):
                        eng = (nc.gpsimd if jc >= NCH - n_pool_c
                               else nc.vector)
                        eng.tensor_scalar(
                            c_all[:, jc, :], r_bc[:],
                            w_sb[:, jc, h:h + 1], p_sb[:, jc, h:h + 1],
                            ALU.max, ALU.mult)
                    # em' = c * adjT (split across DVE/Pool to balance)
                    em_all = attn.tile([128, NCH * 512], BF16, tag="em")
                    cflat = c_all[:].rearrange("p a b -> p (a b)")
                    ch = EM_ENG[h]
                    if ch == 'g':
                        nc.gpsimd.tensor_mul(em_all[:], cflat, adjT_sb[:])
                    elif ch == 'v':
                        nc.vector.tensor_mul(em_all[:], cflat, adjT_sb[:])
                    else:
                        cut = int(ch) * 256
                        nc.vector.tensor_mul(em_all[:, 0:cut],
                                             cflat[:, 0:cut],
                                             adjT_sb[:, 0:cut])
                        nc.gpsimd.tensor_mul(em_all[:, cut:2048],
                                             cflat[:, cut:2048],
                                             adjT_sb[:, cut:2048])

                    # out1 flipped: [i, 64+den] per ic chunk
                    o65v = ps_o.tile([128, NCH, 65], F32, tag="o65")
                    for ic in range(NCH):
                        for jc in range(NCH):
                            nc.tensor.matmul(
                                o65v[:, ic, :],
                                em_all[:, jc * 512 + ic * 128:
                                       jc * 512 + (ic + 1) * 128],
                                hpx[:, jc, h, :],
                                start=(jc == 0), stop=(jc == NCH - 1))
                    rden = elup.tile([128, NCH], F32, tag="rden")
                    nc.vector.reciprocal(rden[:], o65v[:, :, 64])

                    # elu: m = min(v,0), p = relu(v), v = out1*rden
                    m_t = elup.tile([128, NCH, 64], BF16, tag="m_t")
                    p_t = elup.tile([128, NCH, 64], BF16, tag="p_t")
                    if ELU_ENG[h] in 'abd':
                        # fold the per-partition rden into a scaled copy;
                        # min/max are then 4x-mode SBUF tensor_scalars.
                        # 'b': one DVE tensor_tensor with a stride-0
                        # broadcast view of rden (cheapest op count).
                        v_sb = elup.tile([128, NCH, 64], BF16, tag="v_sb")
                        if ELU_ENG[h] == 'b':
                            nc.vector.tensor_mul(
                                v_sb[:], o65v[:, :, 0:64],
                                rden[:].broadcast_to([128, NCH, 64]))
                        else:
                            for ic in range(NCH):
                                if ELU_ENG[h] == 'a':
                                    nc.scalar.activation(
                                        v_sb[:, ic, :], o65v[:, ic, 0:64],
                                        AF.Copy, scale=rden[:, ic:ic + 1])
                                else:
                                    nc.vector.tensor_scalar(
                                        v_sb[:, ic, :], o65v[:, ic, 0:64],
                                        rden[:, ic:ic + 1], None, ALU.mult)
                        vf = v_sb[:].rearrange("p a b -> p (a b)")
                        nc.vector.tensor_scalar(
                            m_t[:].rearrange("p a b -> p (a b)"), vf,
                            0.0, None, ALU.min)
                        peng = (nc.gpsimd if MPP_ENG[h] == 'g'
                                else nc.vector)
                        peng.tensor_scalar(
                            p_t[:].rearrange("p a b -> p (a b)"), vf,
                            0.0, None, ALU.max)
                    else:
                        for ic in range(NCH):
                            nc.vector.tensor_scalar(
                                m_t[:, ic, :], o65v[:, ic, 0:64],
                                rden[:, ic:ic + 1], 0.0, ALU.mult, ALU.min)
                            nc.vector.tensor_scalar(
                                p_t[:, ic, :], o65v[:, ic, 0:64],
                                rden[:, ic:ic + 1], 0.0, ALU.mult, ALU.max)
                    em_t = elup.tile([128, NCH, 64], BF16, tag="em_t")
                    nc.scalar.activation(
                        em_t[:].rearrange("p a b -> p (a b)"),
                        m_t[:].rearrange("p a b -> p (a b)"), AF.Exp)
                    h2v = elup.tile([128, NCH, 64], BF16, tag="h2v")
                    if AD_ENG[h] == 'g':
                        nc.gpsimd.tensor_add(
                            h2v[:].rearrange("p a b -> p (a b)"),
                            em_t[:].rearrange("p a b -> p (a b)"),
                            p_t[:].rearrange("p a b -> p (a b)"))
                    else:
                        nc.vector.tensor_add(
                            h2v[:].rearrange("p a b -> p (a b)"),
                            em_t[:].rearrange("p a b -> p (a b)"),
                            p_t[:].rearrange("p a b -> p (a b)"))

                    if dbg and g == dbg_g and h == 0 and dbg_lvl >= 3:
                        nc.sync.dma_start(out=dbg_em.ap(), in_=em_all[:])
                        nc.sync.dma_start(out=dbg_h2v.ap(), in_=h2v[:])
                        nc.sync.dma_start(out=dbg_rden.ap(), in_=rden[:])

                    # transpose h2v [i,(ic),64] -> h2cT rows [64, 512]
                    h2T_ps = ps_t.tile([64, 512], BF16, tag="t")
                    for ic in range(NCH):
                        nc.tensor.transpose(
                            h2T_ps[:, ic * 128:(ic + 1) * 128],
                            h2v[:, ic, :], ident[:])
                    prow = (h % 2) * 64
                    if H2_ENG[h] == 'a':
                        nc.scalar.copy(h2cT[prow:prow + 64, h // 2, :],
                                       h2T_ps[:])
                    else:
                        nc.vector.tensor_copy(
                            h2cT[prow:prow + 64, h // 2, :], h2T_ps[:])

                if dbg and g == dbg_g:
                    nc.sync.dma_start(out=dbg_h2cT.ap(), in_=h2cT[:])

                # ================= layer 2 =================
                # h_prime2 (n-major) [i, 16] + ones col -> hp2x bf16
                hp2_ps = ps_o.tile([128, NCH, 65], F32, tag="o65")
                for ic in range(NCH):
                    for fc in range(NCH):
                        nc.tensor.matmul(hp2_ps[:, ic, 0:16],
                                         h2cT[:, fc, ic * 128:(ic + 1) * 128],
                                         w2f_sb[:, fc, :],
                                         start=(fc == 0), stop=False)
                    nc.tensor.matmul(hp2_ps[:, ic, 0:16],
                                     onesrow[:, ic * 128:(ic + 1) * 128],
                                     negcs_sb[:],
                                     start=False, stop=True)
                hp2x = small.tile([128, NCH, 17], BF16, tag="hp2x")
                nc.vector.tensor_copy(hp2x[:, :, 0:16], hp2_ps[:, :, 0:16])
                nc.vector.memset(hp2x[:, :, 16:17], 1.0)

                # h_prime2T [16, n] -> tanh -> t2 bf16
                hp2T_ps = ps_row.tile([16, 512], F32, tag="row")
                for fc in range(NCH):
                    nc.tensor.matmul(hp2T_ps[:], w2f_sb[:, fc, :],
                                     h2cT[:, fc, :],
                                     start=(fc == 0), stop=False)
                nc.tensor.matmul(hp2T_ps[:], negcs_sb[:], onesrow[:],
                                 start=False, stop=True)
                t2_sb = small.tile([16, 512], BF16, tag="t2")
                nc.scalar.activation(t2_sb[:], hp2T_ps[:], AF.Tanh)

                # s2 row -> r2 = exp(-0.8 s2); d2 cols -> w2c, p2c
                s2_ps = ps_row.tile([16, 512], F32, tag="row")
                nc.tensor.matmul(s2_ps[0:1, :], a2s_sb[:], t2_sb[:],
                                 start=True, stop=True)
                r2 = small.tile([1, 512], BF16, tag="r2")
                nc.scalar.activation(r2[:], s2_ps[0:1, :], AF.Exp, scale=-0.8)
                d2_ps = ps_o.tile([128, NCH, 65], F32, tag="o65")
                for jc in range(NCH):
                    nc.tensor.matmul(d2_ps[:, jc, 0:1],
                                     t2_sb[:, jc * 128:(jc + 1) * 128],
                                     a2d_sb[:], start=True, stop=True)
                w2c = small.tile([128, NCH], F32, tag="w2c")
                p2c = small.tile([128, NCH], F32, tag="p2c")
                nc.scalar.activation(w2c[:], d2_ps[:, :, 0], AF.Exp, scale=0.8)
                nc.scalar.activation(p2c[:], d2_ps[:, :, 0], AF.Exp, scale=0.2)

                # r2 broadcast + c2 + em2
                rb2ps = ps_rb.tile([128, 512], F32, tag="rb")
                nc.tensor.matmul(rb2ps[:], onesrow[:, 0:128], r2[:],
                                 start=True, stop=True)
                r2_bc = attn.tile([128, 512], BF16, tag="r_bc")
                nc.scalar.copy(r2_bc[:], rb2ps[:])
                c2_all = attn.tile([128, NCH, 512], BF16, tag="c_all")
                for jc in range(NCH):
                    nc.vector.tensor_scalar(
                        c2_all[:, jc, :], r2_bc[:],
                        w2c[:, jc:jc + 1], p2c[:, jc:jc + 1],
                        ALU.max, ALU.mult)
                em2_all = attn.tile([128, NCH * 512], BF16, tag="em")
                c2flat = c2_all[:].rearrange("p a b -> p (a b)")
                nc.vector.tensor_mul(em2_all[:, 0:1536], c2flat[:, 0:1536],
                                     adjT_sb[:, 0:1536])
                nc.gpsimd.tensor_mul(em2_all[:, 1536:2048],
                                     c2flat[:, 1536:2048],
                                     adjT_sb[:, 1536:2048])

                # out2 flipped [i, 16+den] per ic; logits = out2 * rden2
                o2v = ps_o.tile([128, NCH, 65], F32, tag="o65")
                for ic in range(NCH):
                    for jc in range(NCH):
                        nc.tensor.matmul(
                            o2v[:, ic, 0:17],
                            em2_all[:, jc * 512 + ic * 128:
                                    jc * 512 + (ic + 1) * 128],
                            hp2x[:, jc, :],
                            start=(jc == 0), stop=(jc == NCH - 1))
                rden2 = elup.tile([128, NCH], F32, tag="rden")
                nc.vector.reciprocal(rden2[:], o2v[:, :, 16])
                nc.vector.tensor_mul(
                    logits_all[:, g, :, :], o2v[:, :, 0:16],
                    rden2[:].broadcast_to([128, NCH, 16]))

                # lsm partial: exp + pairwise sums for this graph
                ex = elup.tile([128, NCH, 16], F32, tag="ex")
                nc.scalar.activation(ex[:].rearrange("p a b -> p (a b)"),
                                     logits_all[:, g, :, :].rearrange(
                                         "p a b -> p (a b)"), AF.Exp)
                s8 = elup.tile([128, NCH, 8], F32, tag="s8")
                nc.vector.tensor_add(s8[:], ex[:, :, 0:8], ex[:, :, 8:16])
                s4 = elup.tile([128, NCH, 4], F32, tag="s4")
                nc.vector.tensor_add(s4[:], s8[:, :, 0:4], s8[:, :, 4:8])
                s2t = elup.tile([128, NCH, 2], F32, tag="s2t")
                nc.vector.tensor_add(s2t[:], s4[:, :, 0:2], s4[:, :, 2:4])
                nc.vector.tensor_add(s1_all[:, g, :], s2t[:, :, 0],
                                     s2t[:, :, 1])

                if dbg and g == dbg_g:
                    nc.sync.dma_start(out=dbg_lg.ap(),
                                      in_=logits_all[:, dbg_g, :, :])

            # ---------- deferred log_softmax (one Ln table switch) ------
            lse = finp.tile([128, GPC, NCH], F32)
            nc.scalar.activation(lse[:], s1_all[:], AF.Ln)
            fin = finp.tile([128, GPC, NCH, 16], F32)
            nc.vector.tensor_sub(
                fin[:], logits_all[:],
                lse[:].broadcast_to([128, GPC, NCH, 16]))
            nc.sync.dma_start(
                out=out_d.ap().rearrange("g (c p) k -> p g c k", c=NCH),
                in_=fin[:])
    return nc


def host_prep(adj, vertices, local_emb, emb0, emb1, w1, a_src1, a_dst1,
              w2, a_src2, a_dst2):
    """Build the 8 per-core input maps from full inputs."""
    adj = np.asarray(adj, dtype=np.float32)
    vertices = np.asarray(vertices)
    local_emb = np.asarray(local_emb, dtype=np.float32)
    emb0 = np.asarray(emb0, dtype=np.float32)
    emb1 = np.asarray(emb1, dtype=np.float32)
    w1 = np.asarray(w1, dtype=np.float32)
    a_src1 = np.asarray(a_src1, dtype=np.float32)
    a_dst1 = np.asarray(a_dst1, dtype=np.float32)
    w2 = np.asarray(w2, dtype=np.float32)
    a_src2 = np.asarray(a_src2, dtype=np.float32)
    a_dst2 = np.asarray(a_dst2, dtype=np.float32)

    hp_np = BFNP if HP_DT == mybir.dt.bfloat16 else np.float32
    vtx = vertices.astype(np.int64)
    # h: [b, n, 160] -> hT [b, 160, n]
    h = np.concatenate([emb0[vtx], emb1[vtx], local_emb], axis=2)
    hT = np.ascontiguousarray(h.transpose(0, 2, 1)).astype(hp_np)

    # adjT packed: [b, 128, 4*512] bf16, block jc = adjT rows jc*128..
    adjT = adj.transpose(0, 2, 1)
    adjTp = np.ascontiguousarray(
        adjT.reshape(BS, NCH, 128, N).transpose(0, 2, 1, 3).reshape(
            BS, 128, NCH * N)).astype(BFNP)

    w1f = np.ascontiguousarray(
        w1.transpose(1, 0, 2).reshape(FIN1, 512)).astype(hp_np)
    asrc = np.zeros((512, NH), np.float32)
    adst = np.zeros((512, NH), np.float32)
    for hh in range(NH):
        asrc[hh * 64:(hh + 1) * 64, hh] = a_src1[hh, :, 0]
        adst[hh * 64:(hh + 1) * 64, hh] = a_dst1[hh, :, 0]
    consts = {
        "w1f": w1f,
        "asrc": asrc.astype(BFNP),
        "adst": adst.astype(BFNP),
        "w2f": w2[0].astype(BFNP),
        "negcs": (-w2[0].sum(axis=0, keepdims=True)).astype(BFNP),
        "a2s": a_src2[0].astype(BFNP),
        "a2d": a_dst2[0].astype(BFNP),
        "ident": np.eye(128, dtype=np.float32).astype(BFNP),
        "bc8": np.repeat(np.eye(NH, dtype=np.float32), 128,
                         axis=1).astype(BFNP),
    }
    in_maps = []
    for core in range(NCORES):
        sl = slice(core * GPC, (core + 1) * GPC)
        m = dict(consts)
        m["hT4"] = np.ascontiguousarray(hT[sl])
        m["adjTp"] = np.ascontiguousarray(adjTp[sl])
        in_maps.append(m)
    return in_maps


_NC_CACHE = {}


def _get_nc():
    if "nc" not in _NC_CACHE:
        nc = build_nc()
        nc.compile()
        _NC_CACHE["nc"] = nc
    return _NC_CACHE["nc"]


def kernel(**inputs):
    from concourse.bass_utils import run_bass_kernel_spmd
    nc = _get_nc()
    in_maps = host_prep(**inputs)
    res = run_bass_kernel_spmd(nc, in_maps, core_ids=list(range(NCORES)))
    out = np.concatenate([r["out"] for r in res.results], axis=0)
    return out.astype(np.float32)


if __name__ == "__main__":
    nc = build_nc()
    print("built ok")
